# revision 1
# baseline (speedup 1.0000x reference)
"""Self-contained Trainium2 Bass kernel for the 4-layer SplineConv GNN.

kernel(**inputs) takes the FULL unsharded inputs (x, pseudo, edge_index,
batch, W1..W4, root1..4, b1..4, fc_w, fc_b) and returns log_softmax logits
[512, 6] float32, computed on 8 NeuronCores (node/edge partition by dst
range, per-core PE-column packing, AllGather of features per layer).
"""
import numpy as np
import concourse.bass as bass
import concourse.bacc as bacc
import concourse.mybir as mybir
import concourse.tile as tile
from concourse.bass_utils import run_bass_kernel_spmd


N_CORES = 8
N_NODES = 80000
N_GRAPHS = 512
NPC = N_NODES // N_CORES          # nodes per core (10000)
NPCOL = 7                         # nodes per column
SLOTS = 128                       # contraction slots per column
BANK_COLS = 32                    # columns per PSUM bank group (32*14=448<=512)


def build_plan(edge_index, pseudo, batch):
    src = np.asarray(edge_index[0], dtype=np.int64)
    dst = np.asarray(edge_index[1], dtype=np.int64)
    u = np.asarray(pseudo, dtype=np.float32).reshape(-1)
    batch = np.asarray(batch, dtype=np.int64)
    E = src.shape[0]

    deg = np.bincount(dst, minlength=N_NODES).astype(np.int64)
    deg_clip = np.maximum(deg, 1).astype(np.float32)

    # sort edges by dst for per-node grouping
    order = np.argsort(dst, kind="stable")
    s_src, s_dst, s_u = src[order], dst[order], u[order]
    rowptr = np.zeros(N_NODES + 1, dtype=np.int64)
    np.cumsum(deg, out=rowptr[1:])

    # --- per-core column packing (uniform across cores) ---
    # Round-robin over deg-sorted nodes balances column loads near 112.
    ncol_req = -(-NPC // NPCOL)  # 1429
    NCOL = -(-ncol_req // BANK_COLS) * BANK_COLS  # 1440
    NLOC = NCOL * NPCOL               # local node slots per core (10080)
    ZROW = N_CORES * NLOC             # zero row index in tables

    # outputs
    col_node = np.full((N_CORES, NCOL, NPCOL), -1, dtype=np.int64)  # global node id or -1
    perm_row = np.empty(N_NODES, dtype=np.int64)  # global node -> table row

    for c in range(N_CORES):
        nodes = np.arange(c * NPC, (c + 1) * NPC)
        nd = deg[nodes]
        sorted_nodes = nodes[np.argsort(-nd, kind="stable")]
        # round-robin deal into NCOL columns
        for i, g in enumerate(sorted_nodes):
            col = i % NCOL
            pos = i // NCOL
            col_node[c, col, pos] = g
        # fix overloaded columns (load > SLOTS) by swapping with lightest
        loads = np.zeros(NCOL, dtype=np.int64)
        for col in range(NCOL):
            ns = col_node[c, col]
            loads[col] = deg[ns[ns >= 0]].sum()
        it = 0
        while loads.max() > SLOTS:
            it += 1
            assert it < 20000, "rebalance failed"
            hi = int(loads.argmax())
            lo = int(loads.argmin())
            hi_nodes = col_node[c, hi]
            lo_nodes = col_node[c, lo]
            # swap the heaviest node of hi with lightest real node of lo
            hi_p = int(np.argmax([deg[n] if n >= 0 else -1 for n in hi_nodes]))
            lo_p = int(np.argmin([deg[n] if n >= 0 else 1 << 30 for n in lo_nodes]))
            a, b = hi_nodes[hi_p], lo_nodes[lo_p]
            if a < 0 or b < 0 or deg[a] <= deg[b]:
                raise RuntimeError("rebalance stuck")
            col_node[c, hi, hi_p], col_node[c, lo, lo_p] = b, a
            loads[hi] += deg[b] - deg[a]
            loads[lo] += deg[a] - deg[b]
        for col in range(NCOL):
            for pos in range(NPCOL):
                g = col_node[c, col, pos]
                if g >= 0:
                    perm_row[g] = c * NLOC + col * NPCOL + pos

    # --- gather idx + patterns ---
    idx_g = np.full((N_CORES, SLOTS, NCOL), ZROW, dtype=np.int32)
    uvals = np.zeros((N_CORES, SLOTS, NCOL), dtype=np.float32)
    pat = np.zeros((N_CORES, SLOTS, NCOL, 2 * NPCOL), dtype=np.float32)
    for c in range(N_CORES):
        for col in range(NCOL):
            p = 0
            for pos in range(NPCOL):
                g = col_node[c, col, pos]
                if g < 0:
                    continue
                lo, hi = rowptr[g], rowptr[g + 1]
                n_e = hi - lo
                assert p + n_e <= SLOTS
                if n_e == 0:
                    continue
                erange = slice(lo, hi)
                sl = slice(p, p + n_e)
                idx_g[c, sl, col] = perm_row[s_src[erange]]
                dinv = np.float32(1.0) / deg_clip[g]
                pat[c, sl, col, 2 * pos] = dinv
                pat[c, sl, col, 2 * pos + 1] = s_u[erange] * dinv
                uvals[c, sl, col] = s_u[erange]
                p += n_e

    # --- per-node metadata in local order ---
    deg_inv = np.zeros((N_CORES, 1, NLOC), dtype=np.float32)
    batch_loc = np.full((N_CORES, NLOC), N_GRAPHS, dtype=np.float32)  # dummy -> 512
    for c in range(N_CORES):
        flat = col_node[c].reshape(-1)  # local order
        valid = flat >= 0
        deg_inv[c, 0, valid] = 1.0 / deg_clip[flat[valid]]
        batch_loc[c, valid] = batch[flat[valid]].astype(np.float32)

    cnt = np.bincount(batch, minlength=N_GRAPHS).astype(np.float32)
    cnt_clip = np.maximum(cnt, 1.0)

    return dict(
        NCOL=NCOL, NLOC=NLOC, ZROW=ZROW,
        col_node=col_node, perm_row=perm_row,
        idx_g=idx_g, pat=pat, deg_inv=deg_inv,
        batch_loc=batch_loc, cnt_clip=cnt_clip,
        deg_clip=deg_clip,
    )


def permute_x(x, plan):
    """x [N,3] -> x_table [8*NLOC+pad, 4] in (core,local) row order, padded."""
    NLOC, ZROW = plan["NLOC"], plan["ZROW"]
    xt = np.zeros((ZROW + 1, 4), dtype=np.float32)
    flat = plan["col_node"].reshape(-1)
    valid = flat >= 0
    rows = np.arange(N_CORES * NLOC)[valid]
    xt[rows, :3] = np.asarray(x, dtype=np.float32)[flat[valid]]
    return xt





N_CORES = 8
NCOL = 1440
NPCOL = 7
NLOC = NCOL * NPCOL          # 10080
ZROW = N_CORES * NLOC        # 80640
BG = 32                      # columns per bank group
NBG = NCOL // BG             # 45
PW = 2 * NPCOL               # 14 pattern cols per column
N_GRAPHS = 512
F = 64

f32 = mybir.dt.float32
i32 = mybir.dt.int32
AL = mybir.AluOpType
ACTF = mybir.ActivationFunctionType
AX = mybir.AxisListType

DCH = [(i * 512, min((i + 1) * 512, NLOC)) for i in range((NLOC + 511) // 512)]
TCH = [(i * 128, min((i + 1) * 128, NLOC)) for i in range((NLOC + 127) // 128)]


def build_nc():
    nc = bacc.Bacc("TRN2", target_bir_lowering=False)

    x_table = nc.dram_tensor("x_table", [ZROW + 1, 4], f32, kind="ExternalInput")
    idx_in = nc.dram_tensor("idx", [128, NCOL], i32, kind="ExternalInput")
    pat_in = nc.dram_tensor("pat", [NBG, 128, BG * PW], f32, kind="ExternalInput")
    xT_in = nc.dram_tensor("xT", [4, NLOC], f32, kind="ExternalInput")
    batchv_in = nc.dram_tensor("batchv", [128, len(TCH)], f32, kind="ExternalInput")
    gids_in = nc.dram_tensor("gids", [128, N_GRAPHS], f32, kind="ExternalInput")
    cnt_in = nc.dram_tensor("cnt", [F, N_GRAPHS], f32, kind="ExternalInput")
    fcb_in = nc.dram_tensor("fcb", [128, 8], f32, kind="ExternalInput")
    fcw_in = nc.dram_tensor("fcw", [F, 8], f32, kind="ExternalInput")
    ident_in = nc.dram_tensor("ident", [F, F], f32, kind="ExternalInput")
    wts_in = []
    for l in range(4):
        fin = 4 if l == 0 else F
        wts_in.append((
            nc.dram_tensor(f"wpack_{l}", [fin, 3 * F], f32, kind="ExternalInput"),
            nc.dram_tensor(f"b_{l}", [F, 1], f32, kind="ExternalInput"),
        ))

    out_logits = nc.dram_tensor("out_logits", [N_GRAPHS, 8], f32, kind="ExternalOutput")

    with tile.TileContext(nc) as tc:
        with (
            tc.tile_pool(name="res", bufs=1) as res,
            tc.tile_pool(name="gbuf", bufs=3) as gbuf,
            tc.tile_pool(name="pbuf", bufs=3) as pbuf,
            tc.tile_pool(name="xbuf", bufs=2) as xbuf,
            tc.tile_pool(name="work", bufs=2) as work,
            tc.tile_pool(name="stage", bufs=3) as stpool,
            tc.tile_pool(name="psum_s", bufs=3, space="PSUM") as ps_s,
            tc.tile_pool(name="psum_d", bufs=2, space="PSUM") as ps_d,
            tc.tile_pool(name="psum_t", bufs=2, space="PSUM") as ps_t,
            tc.tile_pool(name="psum_p", bufs=1, space="PSUM") as ps_p,
            tc.tile_pool(name="dram", bufs=1, space="DRAM") as dr,
        ):
            idx_sb = res.tile([128, NCOL], i32)
            nc.sync.dma_start(idx_sb[:], idx_in[:])
            gids_sb = res.tile([128, N_GRAPHS], f32)
            nc.sync.dma_start(gids_sb[:], gids_in[:])
            batchv_sb = res.tile([128, len(TCH)], f32)
            nc.sync.dma_start(batchv_sb[:], batchv_in[:])
            ident = res.tile([F, F], f32)
            nc.sync.dma_start(ident[:], ident_in[:])

            w_sb = []
            for l in range(4):
                fin = 4 if l == 0 else F
                t = res.tile([fin, 3 * F], f32, tag=f"w{l}")
                nc.sync.dma_start(t[:], wts_in[l][0][:])
                b = res.tile([F, 1], f32, tag=f"bb{l}")
                nc.sync.dma_start(b[:], wts_in[l][1][:])
                w_sb.append((t, b))

            S_pl = res.tile([F, NLOC], f32)   # S_plain^T
            S_u = res.tile([F, NLOC], f32)    # S_u^T
            H = res.tile([F, NLOC], f32)      # h^T, updated in place per layer

            h_tabs = [
                nc.dram_tensor(f"h_tab{i}", [ZROW + 1, F], f32,
                               kind="Internal", addr_space="Shared")
                for i in range(2)
            ]
            ag_in = dr.tile([NLOC, F], f32)
            pool_in = dr.tile([F, N_GRAPHS], f32)
            pool_out = dr.tile([F, N_GRAPHS], f32, addr_space="Shared")

            zrow = res.tile([1, F], f32)
            nc.vector.memset(zrow[:], 0.0)
            for t in h_tabs:
                nc.sync.dma_start(t[ZROW:ZROW + 1, :], zrow[:])

            pool_ps = ps_p.tile([F, N_GRAPHS], f32, space="PSUM")

            for l in range(4):
                fin = 4 if l == 0 else F
                w_t, b_t = w_sb[l]
                if l == 0:
                    table = x_table[:]
                    tw = 4
                else:
                    table = h_tabs[(l - 1) % 2][:]
                    tw = F

                # --- scatter: gather + pattern matmuls + evac ---
                for bg in range(NBG):
                    g_t = gbuf.tile([128, BG, tw], f32, tag=f"g{min(l, 1)}")
                    for c2 in range(BG):
                        nc.gpsimd.indirect_dma_start(
                            out=g_t[:, c2, :], out_offset=None, in_=table,
                            in_offset=bass.IndirectOffsetOnAxis(
                                ap=idx_sb[:, bg * BG + c2:bg * BG + c2 + 1], axis=0),
                        )
                    p_t = pbuf.tile([128, BG * PW], f32, tag="pat")
                    nc.sync.dma_start(p_t[:], pat_in[bg, :, :])
                    bank = ps_s.tile([fin, BG * PW], f32, tag="scat", space="PSUM")
                    for c in range(BG):
                        nc.tensor.matmul(
                            bank[:, c * PW:(c + 1) * PW],
                            lhsT=g_t[:, c, :],
                            rhs=p_t[:, c * PW:(c + 1) * PW],
                            start=True, stop=True,
                        )
                    bview = bank[:].rearrange("f (x two) -> f two x", two=2)
                    dst = slice(bg * BG * NPCOL, (bg + 1) * BG * NPCOL)
                    nc.vector.tensor_copy(S_pl[0:fin, dst], bview[:, 0, :])
                    nc.vector.tensor_copy(S_u[0:fin, dst], bview[:, 1, :])

                # --- dense + ELU -> H (in place) ---
                for (c0, c1) in DCH:
                    n = c1 - c0
                    d_ps = ps_d.tile([F, 512], f32, tag="dense", space="PSUM")
                    nc.tensor.matmul(
                        d_ps[:, 0:n], lhsT=w_t[:, 0:F],
                        rhs=S_pl[0:fin, c0:c1], start=True, stop=False)
                    nc.tensor.matmul(
                        d_ps[:, 0:n], lhsT=w_t[:, F:2 * F],
                        rhs=S_u[0:fin, c0:c1], start=False, stop=False)
                    if l == 0:
                        hprev = xbuf.tile([4, 512], f32, tag="xc")
                        nc.sync.dma_start(hprev[:, 0:n], xT_in[:, c0:c1])
                        hp_ap = hprev[:, 0:n]
                    else:
                        hp_ap = H[0:F, c0:c1]
                    nc.tensor.matmul(
                        d_ps[:, 0:n], lhsT=w_t[:, 2 * F:3 * F],
                        rhs=hp_ap, start=False, stop=True)
                    # ELU(z+b) = relu(z+b) + min(exp(z+b),1) - 1
                    ex_t = work.tile([F, 512], f32, tag="ex")
                    nc.scalar.activation(ex_t[:, 0:n], d_ps[:, 0:n], ACTF.Exp, bias=b_t[:])
                    re_t = work.tile([F, 512], f32, tag="re")
                    nc.scalar.activation(re_t[:, 0:n], d_ps[:, 0:n], ACTF.Relu, bias=b_t[:])
                    nc.vector.tensor_scalar(
                        out=ex_t[:, 0:n], in0=ex_t[:, 0:n],
                        scalar1=1.0, scalar2=-1.0, op0=AL.min, op1=AL.add)
                    nc.vector.tensor_tensor(
                        out=H[0:F, c0:c1], in0=ex_t[:, 0:n],
                        in1=re_t[:, 0:n], op=AL.add)

                # --- transpose to node-major (+ pooling on last layer) ---
                for k, (t0, t1) in enumerate(TCH):
                    n = t1 - t0
                    t_ps = ps_t.tile([128, F], f32, tag="tr", space="PSUM")
                    nc.tensor.transpose(
                        out=t_ps[0:n, :], in_=H[0:F, t0:t1], identity=ident[:])
                    st_t = stpool.tile([128, F], f32, tag="st")
                    nc.vector.tensor_copy(st_t[0:n, :], t_ps[0:n, :])
                    if l < 3:
                        nc.sync.dma_start(ag_in[t0:t1, :], st_t[0:n, :])
                    else:
                        if n < 128:
                            nc.vector.memset(st_t[n:128, :], 0.0)
                        oh_t = work.tile([128, N_GRAPHS], f32, tag="oh")
                        nc.vector.tensor_scalar(
                            out=oh_t[:], in0=gids_sb[:],
                            scalar1=batchv_sb[:, k:k + 1], scalar2=None,
                            op0=AL.is_equal)
                        nc.tensor.matmul(
                            pool_ps[:], lhsT=st_t[:], rhs=oh_t[:],
                            start=(k == 0), stop=(k == len(TCH) - 1))

                if l < 3:
                    nc.gpsimd.collective_compute(
                        "AllGather", AL.bypass,
                        replica_groups=[list(range(N_CORES))],
                        ins=[ag_in.opt()],
                        outs=[h_tabs[l % 2][0:ZROW, :].opt()],
                    )

            # ---------------- pooling all-reduce + head ----------------
            pool_sb = res.tile([F, N_GRAPHS], f32)
            nc.vector.tensor_copy(pool_sb[:], pool_ps[:])
            nc.sync.dma_start(pool_in[:], pool_sb[:])
            nc.gpsimd.collective_compute(
                "AllReduce", AL.add,
                replica_groups=[list(range(N_CORES))],
                ins=[pool_in.opt()], outs=[pool_out.opt()],
            )
            pooled = res.tile([F, N_GRAPHS], f32)
            nc.sync.dma_start(pooled[:], pool_out[:])
            cnt_sb = res.tile([F, N_GRAPHS], f32)
            nc.sync.dma_start(cnt_sb[:], cnt_in[:])
            nc.vector.reciprocal(cnt_sb[:], cnt_sb[:])
            nc.vector.tensor_tensor(out=pooled[:], in0=pooled[:], in1=cnt_sb[:], op=AL.mult)

            fcw_sb = res.tile([F, 8], f32)
            nc.sync.dma_start(fcw_sb[:], fcw_in[:])
            fcb_sb = res.tile([128, 8], f32)
            nc.sync.dma_start(fcb_sb[:], fcb_in[:])

            for gch in range(N_GRAPHS // 128):
                g0 = gch * 128
                l_ps = ps_d.tile([128, 8], f32, tag="dense", space="PSUM")
                nc.tensor.matmul(
                    l_ps[:, 0:8], lhsT=pooled[:, g0:g0 + 128], rhs=fcw_sb[:],
                    start=True, stop=True)
                z_t = work.tile([128, 8], f32, tag="z")
                nc.vector.tensor_tensor(out=z_t[:], in0=l_ps[:], in1=fcb_sb[:], op=AL.add)
                rm = work.tile([128, 1], f32, tag="rm")
                nc.vector.tensor_reduce(rm[:], z_t[:, 0:6], axis=AX.X, op=AL.max)
                zs = work.tile([128, 8], f32, tag="zs")
                nc.vector.tensor_scalar(
                    out=zs[:], in0=z_t[:], scalar1=rm[:], scalar2=None,
                    op0=AL.subtract)
                e_t = work.tile([128, 8], f32, tag="et")
                nc.scalar.activation(e_t[:, 0:6], zs[:, 0:6], ACTF.Exp)
                sm = work.tile([128, 1], f32, tag="sm")
                nc.vector.tensor_reduce(sm[:], e_t[:, 0:6], axis=AX.X, op=AL.add)
                ln = work.tile([128, 1], f32, tag="ln")
                nc.scalar.activation(ln[:], sm[:], ACTF.Ln)
                oT = work.tile([128, 8], f32, tag="oT")
                nc.vector.tensor_scalar(
                    out=oT[:], in0=zs[:], scalar1=ln[:], scalar2=None,
                    op0=AL.subtract)
                nc.sync.dma_start(out_logits[g0:g0 + 128, :], oT[:])

    nc.compile()
    return nc


def make_in_maps(plan, x, weights):
    xt = np.zeros((ZROW + 1, 4), dtype=np.float32)
    flat = plan["col_node"].reshape(-1)
    valid = flat >= 0
    rows = np.arange(N_CORES * NLOC)[valid]
    xt[rows, :3] = np.asarray(x, dtype=np.float32)[flat[valid]]

    gids = np.broadcast_to(
        np.arange(N_GRAPHS, dtype=np.float32), (128, N_GRAPHS)).copy()
    cnt = np.broadcast_to(plan["cnt_clip"][None, :], (F, N_GRAPHS)).copy()
    fcb = np.zeros((128, 8), dtype=np.float32)
    fcb[:, :6] = np.asarray(weights["fc_b"], dtype=np.float32)
    fcw = np.zeros((F, 8), dtype=np.float32)
    fcw[:, :6] = np.asarray(weights["fc_w"], dtype=np.float32)
    ident = np.eye(F, dtype=np.float32)

    in_maps = []
    for c in range(N_CORES):
        im = {
            "x_table": xt,
            "idx": plan["idx_g"][c],
            "pat": np.ascontiguousarray(
                plan["pat"][c].reshape(128, NBG, BG * PW).transpose(1, 0, 2)),
            "xT": np.ascontiguousarray(xt[c * NLOC:(c + 1) * NLOC, :].T),
            "batchv": np.ascontiguousarray(
                np.pad(plan["batch_loc"][c], (0, len(TCH) * 128 - NLOC),
                       constant_values=N_GRAPHS).reshape(len(TCH), 128).T),
            "gids": gids, "cnt": cnt, "fcb": fcb, "fcw": fcw, "ident": ident,
        }
        for l in range(4):
            fin = 4 if l == 0 else F
            W = np.asarray(weights[f"W{l+1}"], dtype=np.float32)
            root = np.asarray(weights[f"root{l+1}"], dtype=np.float32)
            b = np.asarray(weights[f"b{l+1}"], dtype=np.float32)
            wp = np.zeros((fin, 3 * F), np.float32)
            wp[:W.shape[1], 0:F] = W[0]
            wp[:W.shape[1], F:2 * F] = W[1] - W[0]
            wp[:root.shape[0], 2 * F:3 * F] = root
            im[f"wpack_{l}"] = wp
            im[f"b_{l}"] = b.reshape(F, 1).astype(np.float32)
        in_maps.append(im)
    return in_maps


_NC_CACHE = {}


def kernel(**inputs):
    x = np.asarray(inputs["x"], dtype=np.float32)
    pseudo = np.asarray(inputs["pseudo"], dtype=np.float32)
    edge_index = np.asarray(inputs["edge_index"]).astype(np.int64)
    batch = np.asarray(inputs["batch"]).astype(np.int64)
    weights = {k: np.asarray(inputs[k], dtype=np.float32) for k in
               ["W1", "root1", "b1", "W2", "root2", "b2", "W3", "root3",
                "b3", "W4", "root4", "b4", "fc_w", "fc_b"]}

    plan = build_plan(edge_index, pseudo, batch)
    in_maps = make_in_maps(plan, x, weights)

    if "nc" not in _NC_CACHE:
        _NC_CACHE["nc"] = build_nc()
    nc = _NC_CACHE["nc"]

    res = run_bass_kernel_spmd(nc, in_maps, core_ids=list(range(N_CORES)))
    return np.ascontiguousarray(res.results[0]["out_logits"][:, :6]).astype(np.float32)



# revision 8
# speedup vs baseline: 8.5332x; 8.5332x over previous
"""Self-contained Trainium2 Bass kernel for the 4-layer SplineConv GNN.

kernel(**inputs) takes the FULL unsharded inputs (x, pseudo, edge_index,
batch, W1..W4, root1..4, b1..4, fc_w, fc_b) and returns log_softmax logits
[512, 6] float32, computed on 8 NeuronCores.

Sharding: nodes/edges partitioned by dst range across cores; per-core
column packing (7 nodes x 128 slots per PE column); per-layer AllGather of
node features; AllReduce of pooled per-graph sums.

Upload-minimized: per-edge data is packed into ONE int32 per slot
(17-bit row index | 3-bit in-column position | 12-bit quantized u) and the
spline pattern matrices are reconstructed on-device. The root/bias terms
ride along as "self edges" in reserved slots 121..127, which also lets the
dense matmul emit node-major output directly (no transpose stage).
"""
import numpy as np
import jax

# Persistent executable cache: run_bass_kernel_spmd re-jits per call; without
# this every call re-runs the walrus NEFF packager (~2s). With it, warm calls
# fetch the compiled executable from disk.
jax.config.update("jax_compilation_cache_dir", "/tmp/jax_cc_cache")
jax.config.update("jax_persistent_cache_min_entry_size_bytes", -1)
jax.config.update("jax_persistent_cache_min_compile_time_secs", 0.0)

import concourse.bass as bass
import concourse.bacc as bacc
import concourse.mybir as mybir
import concourse.tile as tile
from concourse.bass_utils import run_bass_kernel_spmd


N_CORES = 8
N_NODES = 80000
N_GRAPHS = 512
NPC = N_NODES // N_CORES     # nodes per core (10000)
NPCOL = 7                    # nodes per column
SLOTS_E = 121                # edge slots per column (121..127 are self slots)
NCOL = 1440                  # columns per core
BG = 24                      # columns per PSUM bank group (24*21=504<=512)
NBG = NCOL // BG             # 60
PWC = 3 * NPCOL              # pattern cols per column (mask, mask*u, self)
NLOC = NCOL * NPCOL          # local node slots per core (10080)
ZROW = N_CORES * NLOC        # zero row index in tables (80640)
F = 64
UQ = 4096.0                  # 12-bit u quantization

f32 = mybir.dt.float32
i32 = mybir.dt.int32
AL = mybir.AluOpType
ACTF = mybir.ActivationFunctionType
AX = mybir.AxisListType

CH = [(i * 128, min((i + 1) * 128, NLOC)) for i in range((NLOC + 127) // 128)]


def build_plan(edge_index, pseudo, batch):
    src = np.asarray(edge_index[0], dtype=np.int64)
    dst = np.asarray(edge_index[1], dtype=np.int64)
    u = np.asarray(pseudo, dtype=np.float32).reshape(-1)
    batch = np.asarray(batch, dtype=np.int64)
    E = src.shape[0]

    deg = np.bincount(dst, minlength=N_NODES).astype(np.int64)
    deg_clip = np.maximum(deg, 1).astype(np.float32)

    # sort edges by dst for per-node grouping
    order = np.argsort(dst, kind="stable")
    s_src, s_dst, s_u = src[order], dst[order], u[order]
    rowptr = np.zeros(N_NODES + 1, dtype=np.int64)
    np.cumsum(deg, out=rowptr[1:])

    # --- per-core column packing: LPT bin packing, capacity 7 nodes/col ---
    import heapq
    col_of = np.empty(N_NODES, dtype=np.int64)
    pos_of = np.empty(N_NODES, dtype=np.int64)
    for c in range(N_CORES):
        nodes = np.arange(c * NPC, (c + 1) * NPC)
        sorted_nodes = nodes[np.argsort(-deg[nodes], kind="stable")]
        heap = [(0, j) for j in range(NCOL)]  # (load, col); cols start empty
        counts = np.zeros(NCOL, dtype=np.int64)
        loads = np.zeros(NCOL, dtype=np.int64)
        spill = []
        degs = deg[sorted_nodes]
        for g, d in zip(sorted_nodes.tolist(), degs.tolist()):
            while True:
                load, j = heapq.heappop(heap)
                if counts[j] < NPCOL:
                    break
            col_of[g] = j
            pos_of[g] = counts[j]
            counts[j] += 1
            loads[j] = load + d
            if counts[j] < NPCOL:
                heapq.heappush(heap, (load + d, j))
        assert loads.max() <= SLOTS_E, f"col overload {loads.max()}"

    core_of = np.arange(N_NODES) // NPC
    perm_row = core_of * NLOC + col_of * NPCOL + pos_of  # global node -> table row

    # --- packed slot table: row | pos<<17 | qu<<20 ---
    EMPTY = np.uint32(ZROW | (7 << 17))
    idxp = np.full((N_CORES, 128, NCOL), EMPTY, dtype=np.uint32)

    # edge slots: per (core,col), nodes at pos 0..6 occupy consecutive slots
    deg_cp = np.zeros((N_CORES, NCOL, NPCOL), dtype=np.int64)
    deg_cp[core_of, col_of, pos_of] = deg
    start_cp = np.cumsum(deg_cp, axis=2) - deg_cp  # exclusive cumsum over pos
    slot_start = start_cp[core_of, col_of, pos_of]  # per node

    e_idx = np.arange(E, dtype=np.int64)
    within = e_idx - rowptr[s_dst]
    e_slot = slot_start[s_dst] + within
    e_core = core_of[s_dst]
    e_col = col_of[s_dst]
    qu = np.minimum(np.rint(s_u * UQ), UQ - 1).astype(np.uint32)
    packed = perm_row[s_src].astype(np.uint32) \
        | (pos_of[s_dst].astype(np.uint32) << 17) | (qu << 20)
    idxp[e_core, e_slot, e_col] = packed

    # self slots: slot 121+p gathers node's own row (pos=7, u=0 -> only the
    # constant self pattern column reads it)
    idxp[core_of, SLOTS_E + pos_of, col_of] = \
        perm_row.astype(np.uint32) | np.uint32(7 << 17)

    # --- per-node metadata in node-major chunk layout [128, n_chunks] ---
    nch = len(CH)
    deg_nm = np.zeros((N_CORES, 128 * nch), dtype=np.float32)
    batch_nm = np.full((N_CORES, 128 * nch), float(N_GRAPHS), dtype=np.float32)
    loc_row = col_of * NPCOL + pos_of
    deg_nm[core_of, loc_row] = 1.0 / deg_clip
    batch_nm[core_of, loc_row] = batch.astype(np.float32)
    deg_nm = deg_nm.reshape(N_CORES, nch, 128).transpose(0, 2, 1)
    batch_nm = batch_nm.reshape(N_CORES, nch, 128).transpose(0, 2, 1)

    # --- x table rows in local order, 4th channel = 1 (bias carrier) ---
    cnt = np.bincount(batch, minlength=N_GRAPHS).astype(np.float32)
    cnt_inv = (1.0 / np.maximum(cnt, 1.0)).astype(np.float32)

    return dict(idxp=idxp.view(np.int32), perm_row=perm_row,
                deg_nm=np.ascontiguousarray(deg_nm),
                batch_nm=np.ascontiguousarray(batch_nm),
                cnt_inv=cnt_inv, loc_row=loc_row, core_of=core_of)


def build_nc():
    nc = bacc.Bacc("TRN2", target_bir_lowering=False)

    idxp_in = nc.dram_tensor("idxp", [128, NCOL], i32, kind="ExternalInput")
    xloc_in = nc.dram_tensor("xloc", [NLOC, 4], f32, kind="ExternalInput")
    deg_in = nc.dram_tensor("degnm", [128, len(CH)], f32, kind="ExternalInput")
    batchv_in = nc.dram_tensor("batchv", [128, len(CH)], f32, kind="ExternalInput")
    cntinv_in = nc.dram_tensor("cntinv", [128, 4], f32, kind="ExternalInput")
    fcw_in = nc.dram_tensor("fcw", [F, 8], f32, kind="ExternalInput")
    fcb_in = nc.dram_tensor("fcb", [128, 8], f32, kind="ExternalInput")
    wts_in = []
    for l in range(4):
        rows = 4 if l == 0 else F + 1
        wts_in.append(nc.dram_tensor(f"wpack_{l}", [rows, 3 * F], f32,
                                     kind="ExternalInput"))

    out_logits = nc.dram_tensor("out_logits", [N_GRAPHS, 8], f32,
                                kind="ExternalOutput")

    with tile.TileContext(nc) as tc:
        with (
            tc.tile_pool(name="res", bufs=1) as res,
            tc.tile_pool(name="gbuf", bufs=3) as gbuf,
            tc.tile_pool(name="pbuf", bufs=2) as pbuf,
            tc.tile_pool(name="work", bufs=2) as work,
            tc.tile_pool(name="psum_s", bufs=3, space="PSUM") as ps_s,
            tc.tile_pool(name="psum_d", bufs=2, space="PSUM") as ps_d,
            tc.tile_pool(name="psum_p", bufs=1, space="PSUM") as ps_p,
            tc.tile_pool(name="dram", bufs=1, space="DRAM") as dr,
        ):
            # ---------------- unpack slot table ----------------
            idxp_sb = res.tile([128, NCOL], i32)
            nc.sync.dma_start(idxp_sb[:], idxp_in[:])
            idx_sb = res.tile([128, NCOL], i32)
            nc.vector.tensor_scalar(out=idx_sb[:], in0=idxp_sb[:],
                                    scalar1=0x1FFFF, scalar2=None,
                                    op0=AL.bitwise_and)
            tmp_i = work.tile([128, NCOL], i32, tag="unp")
            nc.vector.tensor_scalar(out=tmp_i[:], in0=idxp_sb[:],
                                    scalar1=17, scalar2=7,
                                    op0=AL.logical_shift_right,
                                    op1=AL.bitwise_and)
            pos_f = res.tile([128, NCOL], f32)
            nc.vector.tensor_copy(pos_f[:], tmp_i[:])
            tmp_i2 = work.tile([128, NCOL], i32, tag="unp")
            nc.vector.tensor_scalar(out=tmp_i2[:], in0=idxp_sb[:],
                                    scalar1=20, scalar2=None,
                                    op0=AL.logical_shift_right)
            u_f = res.tile([128, NCOL], f32)
            nc.vector.tensor_copy(u_f[:], tmp_i2[:])
            nc.vector.tensor_scalar(out=u_f[:], in0=u_f[:], scalar1=1.0 / UQ,
                                    scalar2=None, op0=AL.mult)

            # ---------------- constants built on device ----------------
            iota7_i = res.tile([128, NPCOL], i32)
            nc.gpsimd.iota(iota7_i[:], pattern=[[1, NPCOL]], base=0,
                           channel_multiplier=0)
            iota7 = res.tile([128, NPCOL], f32)
            nc.vector.tensor_copy(iota7[:], iota7_i[:])
            selfp_i = res.tile([128, NPCOL], i32)
            nc.gpsimd.iota(selfp_i[:], pattern=[[-1, NPCOL]], base=-SLOTS_E,
                           channel_multiplier=1)
            selfpat = res.tile([128, NPCOL], f32)
            nc.vector.tensor_scalar(out=selfpat[:], in0=selfp_i[:],
                                    scalar1=0, scalar2=None, op0=AL.is_equal)
            gids_i = res.tile([128, N_GRAPHS], i32)
            nc.gpsimd.iota(gids_i[:], pattern=[[1, N_GRAPHS]], base=0,
                           channel_multiplier=0)
            gids_f = res.tile([128, N_GRAPHS], f32)
            nc.vector.tensor_copy(gids_f[:], gids_i[:])

            # ---------------- small inputs ----------------
            deg_sb = res.tile([128, len(CH)], f32)
            nc.sync.dma_start(deg_sb[:], deg_in[:])
            batchv_sb = res.tile([128, len(CH)], f32)
            nc.sync.dma_start(batchv_sb[:], batchv_in[:])
            cntinv_sb = res.tile([128, 4], f32)
            nc.sync.dma_start(cntinv_sb[:], cntinv_in[:])
            fcw_sb = res.tile([F, 8], f32)
            nc.sync.dma_start(fcw_sb[:], fcw_in[:])
            fcb_sb = res.tile([128, 8], f32)
            nc.sync.dma_start(fcb_sb[:], fcb_in[:])
            w_sb = []
            for l in range(4):
                rows = 4 if l == 0 else F + 1
                t = res.tile([rows, 3 * F], f32, tag=f"w{l}")
                nc.sync.dma_start(t[:], wts_in[l][:])
                w_sb.append(t)

            # ---------------- aggregate buffers ----------------
            S_pl = res.tile([F, NLOC], f32)       # sum_j h_j        (transposed)
            S_u = res.tile([F, NLOC], f32)        # sum_j h_j * u    (transposed)
            S_rt = res.tile([F + 1, NLOC], f32)   # h_i (self); row F = ones
            nc.vector.memset(S_rt[F:F + 1, :], 1.0)

            x_tab = nc.dram_tensor("x_tab", [ZROW + 1, 4], f32,
                                   kind="Internal", addr_space="Shared")
            h_tabs = [
                nc.dram_tensor(f"h_tab{i}", [ZROW + 1, F], f32,
                               kind="Internal", addr_space="Shared")
                for i in range(2)
            ]
            ag_in = dr.tile([NLOC, F], f32)
            pool_in = dr.tile([F, N_GRAPHS], f32)
            pool_out = dr.tile([F, N_GRAPHS], f32, addr_space="Shared")

            zrow = res.tile([1, F], f32)
            nc.vector.memset(zrow[:], 0.0)
            nc.sync.dma_start(x_tab[ZROW:ZROW + 1, :], zrow[:, 0:4])
            for t in h_tabs:
                nc.sync.dma_start(t[ZROW:ZROW + 1, :], zrow[:])

            # gather x across cores (collectives can't read IO tensors
            # directly -> stage through an Internal DRAM buffer)
            xstage = dr.tile([NLOC, 4], f32)
            nc.sync.dma_start(xstage[:], xloc_in[:])
            nc.gpsimd.collective_compute(
                "AllGather", AL.bypass,
                replica_groups=[list(range(N_CORES))],
                ins=[xstage.opt()],
                outs=[x_tab[0:ZROW, :].opt()],
            )

            pool_ps = ps_p.tile([F, N_GRAPHS], f32, space="PSUM")

            for l in range(4):
                fin = 4 if l == 0 else F
                rr = 4 if l == 0 else F + 1  # root matmul contraction rows
                w_t = w_sb[l]
                if l == 0:
                    table = x_tab[:]
                else:
                    table = h_tabs[(l - 1) % 2][:]

                # --- scatter: gather + on-device pattern + matmuls ---
                for bg in range(NBG):
                    c0 = bg * BG
                    g_t = gbuf.tile([128, BG, fin], f32, tag=f"g{min(l, 1)}")
                    for c in range(BG):
                        nc.gpsimd.indirect_dma_start(
                            out=g_t[:, c, :], out_offset=None, in_=table,
                            in_offset=bass.IndirectOffsetOnAxis(
                                ap=idx_sb[:, c0 + c:c0 + c + 1], axis=0),
                        )
                    p_t = pbuf.tile([128, BG, NPCOL, 3], f32, tag="pat")
                    nc.vector.tensor_tensor(
                        out=p_t[:, :, :, 0],
                        in0=pos_f[:, c0:c0 + BG].unsqueeze(2)
                            .to_broadcast([128, BG, NPCOL]),
                        in1=iota7[:].unsqueeze(1).to_broadcast([128, BG, NPCOL]),
                        op=AL.is_equal)
                    nc.vector.tensor_tensor(
                        out=p_t[:, :, :, 1],
                        in0=p_t[:, :, :, 0],
                        in1=u_f[:, c0:c0 + BG].unsqueeze(2)
                            .to_broadcast([128, BG, NPCOL]),
                        op=AL.mult)
                    nc.vector.tensor_copy(
                        p_t[:, :, :, 2],
                        selfpat[:].unsqueeze(1).to_broadcast([128, BG, NPCOL]))

                    bank = ps_s.tile([F, BG * PWC], f32, tag="scat",
                                     space="PSUM")
                    for c in range(BG):
                        nc.tensor.matmul(
                            bank[0:fin, c * PWC:(c + 1) * PWC],
                            lhsT=g_t[:, c, :],
                            rhs=p_t[:, c].rearrange("p k t -> p (k t)"),
                            start=True, stop=True,
                        )
                    bview = bank[0:fin].rearrange("f (c k t) -> f t (c k)",
                                                  k=NPCOL, t=3)
                    dst = slice(c0 * NPCOL, (c0 + BG) * NPCOL)
                    nc.vector.tensor_copy(S_pl[0:fin, dst], bview[:, 0, :])
                    nc.vector.tensor_copy(S_u[0:fin, dst], bview[:, 1, :])
                    nc.vector.tensor_copy(S_rt[0:fin, dst], bview[:, 2, :])

                # --- dense (node-major out) + deg scale + ELU ---
                for k, (t0, t1) in enumerate(CH):
                    n = t1 - t0
                    d_ps = ps_d.tile([128, 128], f32, tag="dense", space="PSUM")
                    nc.tensor.matmul(
                        d_ps[0:n, 0:F], lhsT=S_pl[0:fin, t0:t1],
                        rhs=w_t[0:fin, 0:F], start=True, stop=False)
                    nc.tensor.matmul(
                        d_ps[0:n, 0:F], lhsT=S_u[0:fin, t0:t1],
                        rhs=w_t[0:fin, F:2 * F], start=False, stop=True)
                    nc.tensor.matmul(
                        d_ps[0:n, F:2 * F], lhsT=S_rt[0:rr, t0:t1],
                        rhs=w_t[0:rr, 2 * F:3 * F], start=True, stop=True)
                    z_t = work.tile([128, F], f32, tag="z")
                    nc.vector.tensor_scalar(
                        out=z_t[0:n, :], in0=d_ps[0:n, 0:F],
                        scalar1=deg_sb[0:n, k:k + 1], scalar2=None, op0=AL.mult)
                    nc.vector.tensor_tensor(
                        out=z_t[0:n, :], in0=z_t[0:n, :],
                        in1=d_ps[0:n, F:2 * F], op=AL.add)
                    # ELU(z) = relu(z) + min(exp(z),1) - 1
                    ex_t = work.tile([128, F], f32, tag="ex")
                    nc.scalar.activation(ex_t[0:n, :], z_t[0:n, :], ACTF.Exp)
                    re_t = work.tile([128, F], f32, tag="re")
                    nc.scalar.activation(re_t[0:n, :], z_t[0:n, :], ACTF.Relu)
                    nc.vector.tensor_scalar(
                        out=ex_t[0:n, :], in0=ex_t[0:n, :],
                        scalar1=1.0, scalar2=-1.0, op0=AL.min, op1=AL.add)
                    h_t = work.tile([128, F], f32, tag="h")
                    nc.vector.tensor_tensor(
                        out=h_t[0:n, :], in0=ex_t[0:n, :], in1=re_t[0:n, :],
                        op=AL.add)
                    if l < 3:
                        nc.sync.dma_start(ag_in[t0:t1, :], h_t[0:n, :])
                    else:
                        if n < 128:
                            nc.vector.memset(h_t[n:128, :], 0.0)
                        oh_t = work.tile([128, N_GRAPHS], f32, tag="oh")
                        nc.vector.tensor_scalar(
                            out=oh_t[:], in0=gids_f[:],
                            scalar1=batchv_sb[:, k:k + 1], scalar2=None,
                            op0=AL.is_equal)
                        nc.tensor.matmul(
                            pool_ps[:], lhsT=h_t[:], rhs=oh_t[:],
                            start=(k == 0), stop=(k == len(CH) - 1))

                if l < 3:
                    nc.gpsimd.collective_compute(
                        "AllGather", AL.bypass,
                        replica_groups=[list(range(N_CORES))],
                        ins=[ag_in.opt()],
                        outs=[h_tabs[l % 2][0:ZROW, :].opt()],
                    )

            # ---------------- pooling all-reduce + head ----------------
            pool_sb = res.tile([F, N_GRAPHS], f32)
            nc.vector.tensor_copy(pool_sb[:], pool_ps[:])
            nc.sync.dma_start(pool_in[:], pool_sb[:])
            nc.gpsimd.collective_compute(
                "AllReduce", AL.add,
                replica_groups=[list(range(N_CORES))],
                ins=[pool_in.opt()], outs=[pool_out.opt()],
            )
            pooled = res.tile([F, N_GRAPHS], f32)
            nc.sync.dma_start(pooled[:], pool_out[:])

            for gch in range(N_GRAPHS // 128):
                g0 = gch * 128
                l_ps = ps_p.tile([128, 8], f32, tag="head", space="PSUM")
                nc.tensor.matmul(
                    l_ps[:, 0:8], lhsT=pooled[:, g0:g0 + 128], rhs=fcw_sb[:],
                    start=True, stop=True)
                z_t = work.tile([128, 8], f32, tag="hz")
                nc.vector.tensor_scalar(
                    out=z_t[:], in0=l_ps[:],
                    scalar1=cntinv_sb[:, gch:gch + 1], scalar2=None,
                    op0=AL.mult)
                nc.vector.tensor_tensor(out=z_t[:], in0=z_t[:], in1=fcb_sb[:],
                                        op=AL.add)
                rm = work.tile([128, 1], f32, tag="rm")
                nc.vector.tensor_reduce(rm[:], z_t[:, 0:6], axis=AX.X, op=AL.max)
                zs = work.tile([128, 8], f32, tag="zs")
                nc.vector.tensor_scalar(
                    out=zs[:], in0=z_t[:], scalar1=rm[:], scalar2=None,
                    op0=AL.subtract)
                e_t = work.tile([128, 8], f32, tag="et")
                nc.scalar.activation(e_t[:, 0:6], zs[:, 0:6], ACTF.Exp)
                sm = work.tile([128, 1], f32, tag="sm")
                nc.vector.tensor_reduce(sm[:], e_t[:, 0:6], axis=AX.X, op=AL.add)
                ln = work.tile([128, 1], f32, tag="ln")
                nc.scalar.activation(ln[:], sm[:], ACTF.Ln)
                oT = work.tile([128, 8], f32, tag="oT")
                nc.vector.tensor_scalar(
                    out=oT[:], in0=zs[:], scalar1=ln[:], scalar2=None,
                    op0=AL.subtract)
                nc.sync.dma_start(out_logits[g0:g0 + 128, :], oT[:])

    nc.compile()
    return nc


def make_in_maps(plan, x, weights):
    x = np.asarray(x, dtype=np.float32)
    perm_row, loc_row, core_of = plan["perm_row"], plan["loc_row"], plan["core_of"]

    xloc = np.zeros((N_CORES, NLOC, 4), dtype=np.float32)
    xloc[core_of, loc_row, 0:3] = x
    xloc[core_of, loc_row, 3] = 1.0

    fcb = np.zeros((128, 8), dtype=np.float32)
    fcb[:, :6] = np.asarray(weights["fc_b"], dtype=np.float32)
    fcw = np.zeros((F, 8), dtype=np.float32)
    fcw[:, :6] = np.asarray(weights["fc_w"], dtype=np.float32)
    cntinv = np.ascontiguousarray(
        plan["cnt_inv"].reshape(4, 128).T).astype(np.float32)

    wps = []
    for l in range(4):
        W = np.asarray(weights[f"W{l+1}"], dtype=np.float32)
        root = np.asarray(weights[f"root{l+1}"], dtype=np.float32)
        b = np.asarray(weights[f"b{l+1}"], dtype=np.float32)
        rows = 4 if l == 0 else F + 1
        wp = np.zeros((rows, 3 * F), np.float32)
        fin_d = W.shape[1]  # 3 or 64
        wp[:fin_d, 0:F] = W[0]
        wp[:fin_d, F:2 * F] = W[1] - W[0]
        wp[:fin_d, 2 * F:3 * F] = root
        wp[rows - 1, 2 * F:3 * F] = b  # bias rides the ones row
        wps.append(wp)

    in_maps = []
    for c in range(N_CORES):
        im = {
            "idxp": plan["idxp"][c],
            "xloc": xloc[c],
            "degnm": plan["deg_nm"][c],
            "batchv": plan["batch_nm"][c],
            "cntinv": cntinv,
            "fcw": fcw, "fcb": fcb,
        }
        for l in range(4):
            im[f"wpack_{l}"] = wps[l]
        in_maps.append(im)
    return in_maps


_NC_CACHE = {}


def kernel(**inputs):
    x = np.asarray(inputs["x"], dtype=np.float32)
    pseudo = np.asarray(inputs["pseudo"], dtype=np.float32)
    edge_index = np.asarray(inputs["edge_index"]).astype(np.int64)
    batch = np.asarray(inputs["batch"]).astype(np.int64)
    weights = {k: np.asarray(inputs[k], dtype=np.float32) for k in
               ["W1", "root1", "b1", "W2", "root2", "b2", "W3", "root3",
                "b3", "W4", "root4", "b4", "fc_w", "fc_b"]}

    plan = build_plan(edge_index, pseudo, batch)
    in_maps = make_in_maps(plan, x, weights)

    if "nc" not in _NC_CACHE:
        _NC_CACHE["nc"] = build_nc()
    nc = _NC_CACHE["nc"]

    res = run_bass_kernel_spmd(nc, in_maps, core_ids=list(range(N_CORES)))
    return np.ascontiguousarray(res.results[0]["out_logits"][:, :6]).astype(np.float32)


# revision 12
# speedup vs baseline: 17.3770x; 2.0364x over previous
"""Self-contained Trainium2 Bass kernel for the 4-layer SplineConv GNN.

kernel(**inputs) takes the FULL unsharded inputs (x, pseudo, edge_index,
batch, W1..W4, root1..4, b1..4, fc_w, fc_b) and returns log_softmax logits
[512, 6] float32, computed on 8 NeuronCores.

Sharding: nodes/edges partitioned by dst range across cores; per-core
column packing (7 nodes x 128 slots per PE column); per-layer AllGather of
node features; AllReduce of pooled per-graph sums.

Upload-minimized: per-edge data is packed into ONE int32 per slot
(17-bit row index | 3-bit in-column position | 12-bit quantized u) and the
spline pattern matrices are reconstructed on-device. The root/bias terms
ride along as "self edges" in reserved slots 121..127, which also lets the
dense matmul emit node-major output directly (no transpose stage).
"""
import numpy as np
import jax

# Persistent executable cache: run_bass_kernel_spmd re-jits per call; without
# this every call re-runs the walrus NEFF packager (~2s). With it, warm calls
# fetch the compiled executable from disk.
jax.config.update("jax_compilation_cache_dir", "/tmp/jax_cc_cache")
jax.config.update("jax_persistent_cache_min_entry_size_bytes", -1)
jax.config.update("jax_persistent_cache_min_compile_time_secs", 0.0)

import concourse.bass as bass
import concourse.bacc as bacc
import concourse.mybir as mybir
import concourse.tile as tile
from concourse.bass import ds
from concourse.bass_utils import run_bass_kernel_spmd


N_CORES = 8
N_NODES = 80000
N_GRAPHS = 512
NPC = N_NODES // N_CORES     # nodes per core (10000)
NPCOL = 7                    # nodes per column
SLOTS_E = 121                # edge slots per column (121..127 are self slots)
NCOL = 1440                  # columns per core
BG = 24                      # columns per PSUM bank group (24*21=504<=512)
NBG = NCOL // BG             # 60
PWC = 3 * NPCOL              # pattern cols per column (mask, mask*u, self)
NLOC = NCOL * NPCOL          # local node slots per core (10080)
ZROW = N_CORES * NLOC        # zero row index in tables (80640)
F = 64
UQ = 4096.0                  # 12-bit u quantization

f32 = mybir.dt.float32
i32 = mybir.dt.int32
AL = mybir.AluOpType
ACTF = mybir.ActivationFunctionType
AX = mybir.AxisListType

CH = [(i * 128, min((i + 1) * 128, NLOC)) for i in range((NLOC + 127) // 128)]


def build_plan(edge_index, pseudo, batch):
    src = np.asarray(edge_index[0], dtype=np.int64)
    dst = np.asarray(edge_index[1], dtype=np.int64)
    u = np.asarray(pseudo, dtype=np.float32).reshape(-1)
    batch = np.asarray(batch, dtype=np.int64)
    E = src.shape[0]

    deg = np.bincount(dst, minlength=N_NODES).astype(np.int64)
    deg_clip = np.maximum(deg, 1).astype(np.float32)

    # sort edges by dst for per-node grouping
    order = np.argsort(dst, kind="stable")
    s_src, s_dst, s_u = src[order], dst[order], u[order]
    rowptr = np.zeros(N_NODES + 1, dtype=np.int64)
    np.cumsum(deg, out=rowptr[1:])

    # --- per-core column packing: LPT bin packing, capacity 7 nodes/col ---
    import heapq
    col_of = np.empty(N_NODES, dtype=np.int64)
    pos_of = np.empty(N_NODES, dtype=np.int64)
    for c in range(N_CORES):
        nodes = np.arange(c * NPC, (c + 1) * NPC)
        sorted_nodes = nodes[np.argsort(-deg[nodes], kind="stable")]
        heap = [(0, j) for j in range(NCOL)]  # (load, col); cols start empty
        counts = np.zeros(NCOL, dtype=np.int64)
        loads = np.zeros(NCOL, dtype=np.int64)
        spill = []
        degs = deg[sorted_nodes]
        for g, d in zip(sorted_nodes.tolist(), degs.tolist()):
            while True:
                load, j = heapq.heappop(heap)
                if counts[j] < NPCOL:
                    break
            col_of[g] = j
            pos_of[g] = counts[j]
            counts[j] += 1
            loads[j] = load + d
            if counts[j] < NPCOL:
                heapq.heappush(heap, (load + d, j))
        assert loads.max() <= SLOTS_E, f"col overload {loads.max()}"

    core_of = np.arange(N_NODES) // NPC
    perm_row = core_of * NLOC + col_of * NPCOL + pos_of  # global node -> table row

    # --- packed slot table: row | pos<<17 | qu<<20 ---
    EMPTY = np.uint32(ZROW | (7 << 17))
    idxp = np.full((N_CORES, 128, NCOL), EMPTY, dtype=np.uint32)

    # edge slots: per (core,col), nodes at pos 0..6 occupy consecutive slots
    deg_cp = np.zeros((N_CORES, NCOL, NPCOL), dtype=np.int64)
    deg_cp[core_of, col_of, pos_of] = deg
    start_cp = np.cumsum(deg_cp, axis=2) - deg_cp  # exclusive cumsum over pos
    slot_start = start_cp[core_of, col_of, pos_of]  # per node

    e_idx = np.arange(E, dtype=np.int64)
    within = e_idx - rowptr[s_dst]
    e_slot = slot_start[s_dst] + within
    e_core = core_of[s_dst]
    e_col = col_of[s_dst]
    qu = np.minimum(np.rint(s_u * UQ), UQ - 1).astype(np.uint32)
    packed = perm_row[s_src].astype(np.uint32) \
        | (pos_of[s_dst].astype(np.uint32) << 17) | (qu << 20)
    idxp[e_core, e_slot, e_col] = packed

    # self slots: slot 121+p gathers node's own row (pos=7, u=0 -> only the
    # constant self pattern column reads it)
    idxp[core_of, SLOTS_E + pos_of, col_of] = \
        perm_row.astype(np.uint32) | np.uint32(7 << 17)

    # --- per-node metadata in node-major chunk layout [128, n_chunks] ---
    nch = len(CH)
    deg_nm = np.zeros((N_CORES, 128 * nch), dtype=np.float32)
    batch_nm = np.full((N_CORES, 128 * nch), float(N_GRAPHS), dtype=np.float32)
    loc_row = col_of * NPCOL + pos_of
    deg_nm[core_of, loc_row] = 1.0 / deg_clip
    batch_nm[core_of, loc_row] = batch.astype(np.float32)
    deg_nm = deg_nm.reshape(N_CORES, nch, 128).transpose(0, 2, 1)
    batch_nm = batch_nm.reshape(N_CORES, nch, 128).transpose(0, 2, 1)

    # --- x table rows in local order, 4th channel = 1 (bias carrier) ---
    cnt = np.bincount(batch, minlength=N_GRAPHS).astype(np.float32)
    cnt_inv = (1.0 / np.maximum(cnt, 1.0)).astype(np.float32)

    return dict(idxp=idxp.view(np.int32), perm_row=perm_row,
                deg_nm=np.ascontiguousarray(deg_nm),
                batch_nm=np.ascontiguousarray(batch_nm),
                cnt_inv=cnt_inv, loc_row=loc_row, core_of=core_of)


def build_nc():
    nc = bacc.Bacc("TRN2", target_bir_lowering=False)

    idxp_in = nc.dram_tensor("idxp", [128, NCOL], i32, kind="ExternalInput")
    xloc_in = nc.dram_tensor("xloc", [NLOC, 4], f32, kind="ExternalInput")
    deg_in = nc.dram_tensor("degnm", [128, len(CH)], f32, kind="ExternalInput")
    batchv_in = nc.dram_tensor("batchv", [128, len(CH)], f32, kind="ExternalInput")
    cntinv_in = nc.dram_tensor("cntinv", [128, 4], f32, kind="ExternalInput")
    fcw_in = nc.dram_tensor("fcw", [F, 8], f32, kind="ExternalInput")
    fcb_in = nc.dram_tensor("fcb", [128, 8], f32, kind="ExternalInput")
    wts_in = []
    for l in range(4):
        rows = 4 if l == 0 else F + 1
        wts_in.append(nc.dram_tensor(f"wpack_{l}", [rows, 3 * F], f32,
                                     kind="ExternalInput"))

    out_logits = nc.dram_tensor("out_logits", [N_GRAPHS, 8], f32,
                                kind="ExternalOutput")

    with tile.TileContext(nc) as tc:
        with (
            tc.tile_pool(name="res", bufs=1) as res,
            tc.tile_pool(name="gbuf", bufs=1) as gbuf,
            tc.tile_pool(name="pbuf", bufs=1) as pbuf,
            tc.tile_pool(name="ibuf", bufs=1) as ibuf,
            tc.tile_pool(name="sbuf_st", bufs=1) as stg,
            tc.tile_pool(name="work", bufs=2) as work,
            tc.tile_pool(name="psum_s", bufs=1, space="PSUM") as ps_s,
            tc.tile_pool(name="psum_d", bufs=1, space="PSUM") as ps_d,
            tc.tile_pool(name="psum_p", bufs=1, space="PSUM") as ps_p,
            tc.tile_pool(name="dram", bufs=1, space="DRAM") as dr,
        ):
            # ---------------- unpack slot table ----------------
            idxp_sb = res.tile([128, NCOL], i32)
            nc.sync.dma_start(idxp_sb[:], idxp_in[:])
            idx_sb = res.tile([128, NCOL], i32)
            nc.vector.tensor_scalar(out=idx_sb[:], in0=idxp_sb[:],
                                    scalar1=0x1FFFF, scalar2=None,
                                    op0=AL.bitwise_and)
            tmp_i = work.tile([128, NCOL], i32, tag="unp")
            nc.vector.tensor_scalar(out=tmp_i[:], in0=idxp_sb[:],
                                    scalar1=17, scalar2=7,
                                    op0=AL.logical_shift_right,
                                    op1=AL.bitwise_and)
            pos_f = res.tile([128, NCOL], f32)
            nc.vector.tensor_copy(pos_f[:], tmp_i[:])
            tmp_i2 = work.tile([128, NCOL], i32, tag="unp")
            nc.vector.tensor_scalar(out=tmp_i2[:], in0=idxp_sb[:],
                                    scalar1=20, scalar2=None,
                                    op0=AL.logical_shift_right)
            u_f = res.tile([128, NCOL], f32)
            nc.vector.tensor_copy(u_f[:], tmp_i2[:])
            nc.vector.tensor_scalar(out=u_f[:], in0=u_f[:], scalar1=1.0 / UQ,
                                    scalar2=None, op0=AL.mult)

            # ---------------- constants built on device ----------------
            iota7_i = res.tile([128, NPCOL], i32)
            nc.gpsimd.iota(iota7_i[:], pattern=[[1, NPCOL]], base=0,
                           channel_multiplier=0)
            iota7 = res.tile([128, NPCOL], f32)
            nc.vector.tensor_copy(iota7[:], iota7_i[:])
            selfp_i = res.tile([128, NPCOL], i32)
            nc.gpsimd.iota(selfp_i[:], pattern=[[-1, NPCOL]], base=-SLOTS_E,
                           channel_multiplier=1)
            selfpat = res.tile([128, NPCOL], f32)
            nc.vector.tensor_scalar(out=selfpat[:], in0=selfp_i[:],
                                    scalar1=0, scalar2=None, op0=AL.is_equal)
            gids_i = res.tile([128, N_GRAPHS], i32)
            nc.gpsimd.iota(gids_i[:], pattern=[[1, N_GRAPHS]], base=0,
                           channel_multiplier=0)
            gids_f = res.tile([128, N_GRAPHS], f32)
            nc.vector.tensor_copy(gids_f[:], gids_i[:])

            # ---------------- small inputs ----------------
            deg_sb = res.tile([128, len(CH)], f32)
            nc.sync.dma_start(deg_sb[:], deg_in[:])
            batchv_sb = res.tile([128, len(CH)], f32)
            nc.sync.dma_start(batchv_sb[:], batchv_in[:])
            cntinv_sb = res.tile([128, 4], f32)
            nc.sync.dma_start(cntinv_sb[:], cntinv_in[:])
            fcw_sb = res.tile([F, 8], f32)
            nc.sync.dma_start(fcw_sb[:], fcw_in[:])
            fcb_sb = res.tile([128, 8], f32)
            nc.sync.dma_start(fcb_sb[:], fcb_in[:])
            w_sb = []
            for l in range(4):
                rows = 4 if l == 0 else F + 1
                t = res.tile([rows, 3 * F], f32, tag=f"w{l}")
                nc.sync.dma_start(t[:], wts_in[l][:])
                w_sb.append(t)

            # ---------------- aggregate buffers ----------------
            S_pl = res.tile([F, NLOC], f32)       # sum_j h_j        (transposed)
            S_u = res.tile([F, NLOC], f32)        # sum_j h_j * u    (transposed)
            S_rt = res.tile([F + 1, NLOC], f32)   # h_i (self); row F = ones
            nc.vector.memset(S_rt[F:F + 1, :], 1.0)

            x_tab = nc.dram_tensor("x_tab", [ZROW + 1, 4], f32,
                                   kind="Internal", addr_space="Shared")
            h_tabs = [
                nc.dram_tensor(f"h_tab{i}", [ZROW + 1, F], f32,
                               kind="Internal", addr_space="Shared")
                for i in range(2)
            ]
            ag_in = dr.tile([NLOC, F], f32)
            pool_in = dr.tile([F, N_GRAPHS], f32)
            pool_out = dr.tile([F, N_GRAPHS], f32, addr_space="Shared")

            zrow = res.tile([1, F], f32)
            nc.vector.memset(zrow[:], 0.0)
            nc.sync.dma_start(x_tab[ZROW:ZROW + 1, :], zrow[:, 0:4])
            for t in h_tabs:
                nc.sync.dma_start(t[ZROW:ZROW + 1, :], zrow[:])

            # gather x across cores (collectives can't read IO tensors
            # directly -> stage through an Internal DRAM buffer)
            xstage = dr.tile([NLOC, 4], f32)
            nc.sync.dma_start(xstage[:], xloc_in[:])
            nc.gpsimd.collective_compute(
                "AllGather", AL.bypass,
                replica_groups=[list(range(N_CORES))],
                ins=[xstage.opt()],
                outs=[x_tab[0:ZROW, :].opt()],
            )

            pool_ps = ps_p.tile([F, N_GRAPHS], f32, space="PSUM")

            def scat_body(l, fin, table, c0, uu):
                """One bank group of the scatter stage; c0 may be symbolic."""
                idx_st = ibuf.tile([128, BG], i32, tag=f"ist{uu}")
                nc.vector.tensor_copy(idx_st[:], idx_sb[:, ds(c0, BG)])
                g_t = gbuf.tile([128, BG, fin], f32, tag=f"g{min(l, 1)}_{uu}")
                for c in range(BG):
                    nc.gpsimd.indirect_dma_start(
                        out=g_t[:, c, :], out_offset=None, in_=table,
                        in_offset=bass.IndirectOffsetOnAxis(
                            ap=idx_st[:, c:c + 1], axis=0),
                    )
                p_t = pbuf.tile([128, BG, NPCOL, 3], f32, tag=f"pat{uu}")
                nc.vector.tensor_tensor(
                    out=p_t[:, :, :, 0],
                    in0=pos_f[:, ds(c0, BG)].unsqueeze(2)
                        .to_broadcast([128, BG, NPCOL]),
                    in1=iota7[:].unsqueeze(1).to_broadcast([128, BG, NPCOL]),
                    op=AL.is_equal)
                nc.vector.tensor_tensor(
                    out=p_t[:, :, :, 1],
                    in0=p_t[:, :, :, 0],
                    in1=u_f[:, ds(c0, BG)].unsqueeze(2)
                        .to_broadcast([128, BG, NPCOL]),
                    op=AL.mult)
                nc.vector.tensor_copy(
                    p_t[:, :, :, 2],
                    selfpat[:].unsqueeze(1).to_broadcast([128, BG, NPCOL]))
                bank = ps_s.tile([F, BG * PWC], f32, tag=f"scat{uu}",
                                 space="PSUM")
                for c in range(BG):
                    nc.tensor.matmul(
                        bank[0:fin, c * PWC:(c + 1) * PWC],
                        lhsT=g_t[:, c, :],
                        rhs=p_t[:, c].rearrange("p k t -> p (k t)"),
                        start=True, stop=True,
                    )
                bview = bank[0:fin].rearrange("f (c k t) -> f t (c k)",
                                              k=NPCOL, t=3)
                dst = ds(c0 * NPCOL, BG * NPCOL)
                nc.vector.tensor_copy(S_pl[0:fin, dst], bview[:, 0, :])
                nc.vector.tensor_copy(S_u[0:fin, dst], bview[:, 1, :])
                nc.vector.tensor_copy(S_rt[0:fin, dst], bview[:, 2, :])

            def dense_chunk(l, fin, rr, w_t, t0, n, k_idx, uu,
                            symbolic):
                """Dense + deg scale + ELU for nodes [t0, t0+n)."""
                if symbolic:
                    spl = stg.tile([F, 128], f32, tag=f"spl{uu}")
                    nc.vector.tensor_copy(spl[0:fin, 0:n],
                                          S_pl[0:fin, ds(t0, n)])
                    su = stg.tile([F, 128], f32, tag=f"su{uu}")
                    nc.vector.tensor_copy(su[0:fin, 0:n],
                                          S_u[0:fin, ds(t0, n)])
                    srt = stg.tile([F + 1, 128], f32, tag=f"srt{uu}")
                    nc.vector.tensor_copy(srt[0:rr, 0:n],
                                          S_rt[0:rr, ds(t0, n)])
                    spl_ap, su_ap, srt_ap = (spl[0:fin, 0:n], su[0:fin, 0:n],
                                             srt[0:rr, 0:n])
                    degc = deg_sb[0:n, ds(k_idx, 1)]
                else:
                    spl_ap = S_pl[0:fin, t0:t0 + n]
                    su_ap = S_u[0:fin, t0:t0 + n]
                    srt_ap = S_rt[0:rr, t0:t0 + n]
                    degc = deg_sb[0:n, k_idx:k_idx + 1]
                d_ps = ps_d.tile([128, 128], f32, tag=f"dense{uu}",
                                 space="PSUM")
                nc.tensor.matmul(d_ps[0:n, 0:F], lhsT=spl_ap,
                                 rhs=w_t[0:fin, 0:F], start=True, stop=False)
                nc.tensor.matmul(d_ps[0:n, 0:F], lhsT=su_ap,
                                 rhs=w_t[0:fin, F:2 * F], start=False,
                                 stop=True)
                nc.tensor.matmul(d_ps[0:n, F:2 * F], lhsT=srt_ap,
                                 rhs=w_t[0:rr, 2 * F:3 * F], start=True,
                                 stop=True)
                z_t = work.tile([128, F], f32, tag=f"z{uu}")
                nc.vector.tensor_scalar(
                    out=z_t[0:n, :], in0=d_ps[0:n, 0:F],
                    scalar1=degc, scalar2=None, op0=AL.mult)
                nc.vector.tensor_tensor(
                    out=z_t[0:n, :], in0=z_t[0:n, :],
                    in1=d_ps[0:n, F:2 * F], op=AL.add)
                # ELU(z) = max(z, min(exp(z),1) - 1)
                ex_t = work.tile([128, F], f32, tag=f"ex{uu}")
                nc.scalar.activation(ex_t[0:n, :], z_t[0:n, :], ACTF.Exp)
                nc.vector.tensor_scalar(
                    out=ex_t[0:n, :], in0=ex_t[0:n, :],
                    scalar1=1.0, scalar2=-1.0, op0=AL.min, op1=AL.add)
                h_t = work.tile([128, F], f32, tag=f"h{uu}")
                nc.vector.tensor_tensor(
                    out=h_t[0:n, :], in0=z_t[0:n, :], in1=ex_t[0:n, :],
                    op=AL.max)
                return h_t

            NFULL = (NLOC // 128) * 128  # 9984

            for l in range(4):
                fin = 4 if l == 0 else F
                rr = 4 if l == 0 else F + 1  # root matmul contraction rows
                w_t = w_sb[l]
                if l == 0:
                    table = x_tab[:]
                else:
                    table = h_tabs[(l - 1) % 2][:]

                # --- scatter: gather + on-device pattern + matmuls ---
                with tc.For_i(0, NCOL, BG * 2) as i0:
                    for uu in range(2):
                        scat_body(l, fin, table, i0 + uu * BG, uu)

                # --- dense (node-major out) + deg scale + ELU ---
                if l < 3:
                    with tc.For_i(0, NFULL, 256) as i0:
                        for uu in range(2):
                            t0 = i0 + uu * 128
                            h_t = dense_chunk(l, fin, rr, w_t, t0, 128,
                                              t0 // 128, uu, True)
                            nc.sync.dma_start(ag_in[ds(t0, 128), :], h_t[:])
                    # tail chunk
                    n = NLOC - NFULL
                    h_t = dense_chunk(l, fin, rr, w_t, NFULL, n,
                                      NFULL // 128, 0, False)
                    nc.sync.dma_start(ag_in[NFULL:NLOC, :], h_t[0:n, :])
                    nc.gpsimd.collective_compute(
                        "AllGather", AL.bypass,
                        replica_groups=[list(range(N_CORES))],
                        ins=[ag_in.opt()],
                        outs=[h_tabs[l % 2][0:ZROW, :].opt()],
                    )
                else:
                    for k, (t0, t1) in enumerate(CH):
                        n = t1 - t0
                        h_t = dense_chunk(l, fin, rr, w_t, t0, n, k,
                                          k % 2, False)
                        if n < 128:
                            nc.vector.memset(h_t[n:128, :], 0.0)
                        oh_t = work.tile([128, N_GRAPHS], f32, tag="oh")
                        nc.vector.tensor_scalar(
                            out=oh_t[:], in0=gids_f[:],
                            scalar1=batchv_sb[:, k:k + 1], scalar2=None,
                            op0=AL.is_equal)
                        nc.tensor.matmul(
                            pool_ps[:], lhsT=h_t[:], rhs=oh_t[:],
                            start=(k == 0), stop=(k == len(CH) - 1))

            # ---------------- pooling all-reduce + head ----------------
            pool_sb = res.tile([F, N_GRAPHS], f32)
            nc.vector.tensor_copy(pool_sb[:], pool_ps[:])
            nc.sync.dma_start(pool_in[:], pool_sb[:])
            nc.gpsimd.collective_compute(
                "AllReduce", AL.add,
                replica_groups=[list(range(N_CORES))],
                ins=[pool_in.opt()], outs=[pool_out.opt()],
            )
            pooled = res.tile([F, N_GRAPHS], f32)
            nc.sync.dma_start(pooled[:], pool_out[:])

            for gch in range(N_GRAPHS // 128):
                g0 = gch * 128
                l_ps = ps_p.tile([128, 8], f32, tag="head", space="PSUM")
                nc.tensor.matmul(
                    l_ps[:, 0:8], lhsT=pooled[:, g0:g0 + 128], rhs=fcw_sb[:],
                    start=True, stop=True)
                z_t = work.tile([128, 8], f32, tag="hz")
                nc.vector.tensor_scalar(
                    out=z_t[:], in0=l_ps[:],
                    scalar1=cntinv_sb[:, gch:gch + 1], scalar2=None,
                    op0=AL.mult)
                nc.vector.tensor_tensor(out=z_t[:], in0=z_t[:], in1=fcb_sb[:],
                                        op=AL.add)
                rm = work.tile([128, 1], f32, tag="rm")
                nc.vector.tensor_reduce(rm[:], z_t[:, 0:6], axis=AX.X, op=AL.max)
                zs = work.tile([128, 8], f32, tag="zs")
                nc.vector.tensor_scalar(
                    out=zs[:], in0=z_t[:], scalar1=rm[:], scalar2=None,
                    op0=AL.subtract)
                e_t = work.tile([128, 8], f32, tag="et")
                nc.scalar.activation(e_t[:, 0:6], zs[:, 0:6], ACTF.Exp)
                sm = work.tile([128, 1], f32, tag="sm")
                nc.vector.tensor_reduce(sm[:], e_t[:, 0:6], axis=AX.X, op=AL.add)
                ln = work.tile([128, 1], f32, tag="ln")
                nc.scalar.activation(ln[:], sm[:], ACTF.Ln)
                oT = work.tile([128, 8], f32, tag="oT")
                nc.vector.tensor_scalar(
                    out=oT[:], in0=zs[:], scalar1=ln[:], scalar2=None,
                    op0=AL.subtract)
                nc.sync.dma_start(out_logits[g0:g0 + 128, :], oT[:])

    nc.compile()
    return nc


def make_in_maps(plan, x, weights):
    x = np.asarray(x, dtype=np.float32)
    perm_row, loc_row, core_of = plan["perm_row"], plan["loc_row"], plan["core_of"]

    xloc = np.zeros((N_CORES, NLOC, 4), dtype=np.float32)
    xloc[core_of, loc_row, 0:3] = x
    xloc[core_of, loc_row, 3] = 1.0

    fcb = np.zeros((128, 8), dtype=np.float32)
    fcb[:, :6] = np.asarray(weights["fc_b"], dtype=np.float32)
    fcw = np.zeros((F, 8), dtype=np.float32)
    fcw[:, :6] = np.asarray(weights["fc_w"], dtype=np.float32)
    cntinv = np.ascontiguousarray(
        plan["cnt_inv"].reshape(4, 128).T).astype(np.float32)

    wps = []
    for l in range(4):
        W = np.asarray(weights[f"W{l+1}"], dtype=np.float32)
        root = np.asarray(weights[f"root{l+1}"], dtype=np.float32)
        b = np.asarray(weights[f"b{l+1}"], dtype=np.float32)
        rows = 4 if l == 0 else F + 1
        wp = np.zeros((rows, 3 * F), np.float32)
        fin_d = W.shape[1]  # 3 or 64
        wp[:fin_d, 0:F] = W[0]
        wp[:fin_d, F:2 * F] = W[1] - W[0]
        wp[:fin_d, 2 * F:3 * F] = root
        wp[rows - 1, 2 * F:3 * F] = b  # bias rides the ones row
        wps.append(wp)

    in_maps = []
    for c in range(N_CORES):
        im = {
            "idxp": plan["idxp"][c],
            "xloc": xloc[c],
            "degnm": plan["deg_nm"][c],
            "batchv": plan["batch_nm"][c],
            "cntinv": cntinv,
            "fcw": fcw, "fcb": fcb,
        }
        for l in range(4):
            im[f"wpack_{l}"] = wps[l]
        in_maps.append(im)
    return in_maps


_NC_CACHE = {}


def kernel(**inputs):
    x = np.asarray(inputs["x"], dtype=np.float32)
    pseudo = np.asarray(inputs["pseudo"], dtype=np.float32)
    edge_index = np.asarray(inputs["edge_index"]).astype(np.int64)
    batch = np.asarray(inputs["batch"]).astype(np.int64)
    weights = {k: np.asarray(inputs[k], dtype=np.float32) for k in
               ["W1", "root1", "b1", "W2", "root2", "b2", "W3", "root3",
                "b3", "W4", "root4", "b4", "fc_w", "fc_b"]}

    plan = build_plan(edge_index, pseudo, batch)
    in_maps = make_in_maps(plan, x, weights)

    if "nc" not in _NC_CACHE:
        _NC_CACHE["nc"] = build_nc()
    nc = _NC_CACHE["nc"]

    res = run_bass_kernel_spmd(nc, in_maps, core_ids=list(range(N_CORES)))
    return np.ascontiguousarray(res.results[0]["out_logits"][:, :6]).astype(np.float32)


# revision 23
# speedup vs baseline: 18.2838x; 1.0522x over previous
"""Self-contained Trainium2 Bass kernel for the 4-layer SplineConv GNN.

kernel(**inputs) takes the FULL unsharded inputs (x, pseudo, edge_index,
batch, W1..W4, root1..4, b1..4, fc_w, fc_b) and returns log_softmax logits
[512, 6] float32, computed on 8 NeuronCores.

Sharding: nodes/edges partitioned by dst range across cores; per-core
column packing (7 nodes x 128 slots per PE column); per-layer AllGather of
node features; AllReduce of pooled per-graph sums.

Upload-minimized: per-edge data is packed into ONE int32 per slot
(17-bit row index | 3-bit in-column position | 12-bit quantized u) and the
spline pattern matrices are reconstructed on-device. The root/bias terms
ride along as "self edges" in reserved slots 121..127, which also lets the
dense matmul emit node-major output directly (no transpose stage).
"""
import numpy as np
import jax

# Persistent executable cache: run_bass_kernel_spmd re-jits per call; without
# this every call re-runs the walrus NEFF packager (~2s). With it, warm calls
# fetch the compiled executable from disk.
jax.config.update("jax_compilation_cache_dir", "/tmp/jax_cc_cache")
jax.config.update("jax_persistent_cache_min_entry_size_bytes", -1)
jax.config.update("jax_persistent_cache_min_compile_time_secs", 0.0)

import concourse.bass as bass
import concourse.bacc as bacc
import concourse.mybir as mybir
import concourse.tile as tile
from concourse.bass import ds
from concourse.bass_utils import run_bass_kernel_spmd


N_CORES = 8
N_NODES = 80000
N_GRAPHS = 512
NPC = N_NODES // N_CORES     # nodes per core (10000)
NPCOL = 7                    # nodes per column
SLOTS_E = 121                # edge slots per column (121..127 are self slots)
NCOL = 1440                  # columns per core
BG = 24                      # columns per PSUM bank group (24*21=504<=512)
NBG = NCOL // BG             # 60
PWC = 3 * NPCOL              # pattern cols per column (mask, mask*u, self)
NLOC = NCOL * NPCOL          # local node slots per core (10080)
ZROW = N_CORES * NLOC        # zero row index in tables (80640)
F = 64
UQ = 4096.0                  # 12-bit u quantization

f32 = mybir.dt.float32
bf16 = mybir.dt.bfloat16
i32 = mybir.dt.int32
AL = mybir.AluOpType
ACTF = mybir.ActivationFunctionType
AX = mybir.AxisListType

CH = [(i * 128, min((i + 1) * 128, NLOC)) for i in range((NLOC + 127) // 128)]


def build_plan(edge_index, pseudo, batch):
    src = np.asarray(edge_index[0], dtype=np.int64)
    dst = np.asarray(edge_index[1], dtype=np.int64)
    u = np.asarray(pseudo, dtype=np.float32).reshape(-1)
    batch = np.asarray(batch, dtype=np.int64)
    E = src.shape[0]

    deg = np.bincount(dst, minlength=N_NODES).astype(np.int64)
    deg_clip = np.maximum(deg, 1).astype(np.float32)

    # sort edges by dst for per-node grouping
    order = np.argsort(dst, kind="stable")
    s_src, s_dst, s_u = src[order], dst[order], u[order]
    rowptr = np.zeros(N_NODES + 1, dtype=np.int64)
    np.cumsum(deg, out=rowptr[1:])

    # --- per-core column packing: LPT bin packing, capacity 7 nodes/col ---
    import heapq
    col_of = np.empty(N_NODES, dtype=np.int64)
    pos_of = np.empty(N_NODES, dtype=np.int64)
    for c in range(N_CORES):
        nodes = np.arange(c * NPC, (c + 1) * NPC)
        sorted_nodes = nodes[np.argsort(-deg[nodes], kind="stable")]
        heap = [(0, j) for j in range(NCOL)]  # (load, col); cols start empty
        counts = np.zeros(NCOL, dtype=np.int64)
        loads = np.zeros(NCOL, dtype=np.int64)
        spill = []
        degs = deg[sorted_nodes]
        for g, d in zip(sorted_nodes.tolist(), degs.tolist()):
            while True:
                load, j = heapq.heappop(heap)
                if counts[j] < NPCOL:
                    break
            col_of[g] = j
            pos_of[g] = counts[j]
            counts[j] += 1
            loads[j] = load + d
            if counts[j] < NPCOL:
                heapq.heappush(heap, (load + d, j))
        assert loads.max() <= SLOTS_E, f"col overload {loads.max()}"

    core_of = np.arange(N_NODES) // NPC
    perm_row = core_of * NLOC + col_of * NPCOL + pos_of  # global node -> table row

    # --- packed slot table: row | pos<<17 | qu<<20 ---
    EMPTY = np.uint32(ZROW | (7 << 17))
    idxp = np.full((N_CORES, 128, NCOL), EMPTY, dtype=np.uint32)

    # edge slots: per (core,col), nodes at pos 0..6 occupy consecutive slots
    deg_cp = np.zeros((N_CORES, NCOL, NPCOL), dtype=np.int64)
    deg_cp[core_of, col_of, pos_of] = deg
    start_cp = np.cumsum(deg_cp, axis=2) - deg_cp  # exclusive cumsum over pos
    slot_start = start_cp[core_of, col_of, pos_of]  # per node

    e_idx = np.arange(E, dtype=np.int64)
    within = e_idx - rowptr[s_dst]
    e_slot = slot_start[s_dst] + within
    e_core = core_of[s_dst]
    e_col = col_of[s_dst]
    qu = np.minimum(np.rint(s_u * UQ), UQ - 1).astype(np.uint32)
    packed = perm_row[s_src].astype(np.uint32) \
        | (pos_of[s_dst].astype(np.uint32) << 17) | (qu << 20)
    idxp[e_core, e_slot, e_col] = packed

    # self slots: slot 121+p gathers node's own row (pos=7, u=0 -> only the
    # constant self pattern column reads it)
    idxp[core_of, SLOTS_E + pos_of, col_of] = \
        perm_row.astype(np.uint32) | np.uint32(7 << 17)

    # --- per-node metadata in node-major chunk layout [128, n_chunks] ---
    nch = len(CH)
    deg_nm = np.zeros((N_CORES, 128 * nch), dtype=np.float32)
    batch_nm = np.full((N_CORES, 128 * nch), float(N_GRAPHS), dtype=np.float32)
    loc_row = col_of * NPCOL + pos_of
    deg_nm[core_of, loc_row] = 1.0 / deg_clip
    batch_nm[core_of, loc_row] = batch.astype(np.float32)
    deg_nm = deg_nm.reshape(N_CORES, nch, 128).transpose(0, 2, 1)
    batch_nm = batch_nm.reshape(N_CORES, nch, 128).transpose(0, 2, 1)

    # --- x table rows in local order, 4th channel = 1 (bias carrier) ---
    cnt = np.bincount(batch, minlength=N_GRAPHS).astype(np.float32)
    cnt_inv = (1.0 / np.maximum(cnt, 1.0)).astype(np.float32)

    return dict(idxp=idxp.view(np.int32), perm_row=perm_row,
                deg_nm=np.ascontiguousarray(deg_nm),
                batch_nm=np.ascontiguousarray(batch_nm),
                cnt_inv=cnt_inv, loc_row=loc_row, core_of=core_of)


def build_nc():
    nc = bacc.Bacc("TRN2", target_bir_lowering=False)

    idxp_in = nc.dram_tensor("idxp", [128, NCOL], i32, kind="ExternalInput")
    xloc_in = nc.dram_tensor("xloc", [NLOC, 4], bf16, kind="ExternalInput")
    deg_in = nc.dram_tensor("degnm", [128, len(CH)], f32, kind="ExternalInput")
    batchv_in = nc.dram_tensor("batchv", [128, len(CH)], f32, kind="ExternalInput")
    cntinv_in = nc.dram_tensor("cntinv", [128, 4], f32, kind="ExternalInput")
    fcw_in = nc.dram_tensor("fcw", [F, 8], f32, kind="ExternalInput")
    fcb_in = nc.dram_tensor("fcb", [128, 8], f32, kind="ExternalInput")
    wts_in = []
    for l in range(4):
        rows = 4 if l == 0 else F + 1
        wts_in.append(nc.dram_tensor(f"wpack_{l}", [rows, 3 * F], f32,
                                     kind="ExternalInput"))

    out_logits = nc.dram_tensor("out_logits", [N_GRAPHS, 8], f32,
                                kind="ExternalOutput")

    with tile.TileContext(nc) as tc:
        with (
            tc.tile_pool(name="res", bufs=1) as res,
            tc.tile_pool(name="gbuf", bufs=1) as gbuf,
            tc.tile_pool(name="pbuf", bufs=1) as pbuf,
            tc.tile_pool(name="ibuf", bufs=1) as ibuf,
            tc.tile_pool(name="sbuf_st", bufs=1) as stg,
            tc.tile_pool(name="work", bufs=2) as work,
            tc.tile_pool(name="psum_s", bufs=1, space="PSUM") as ps_s,
            tc.tile_pool(name="psum_d", bufs=1, space="PSUM") as ps_d,
            tc.tile_pool(name="psum_p", bufs=1, space="PSUM") as ps_p,
            tc.tile_pool(name="dram", bufs=1, space="DRAM") as dr,
        ):
            # ---------------- unpack slot table ----------------
            idxp_sb = res.tile([128, NCOL], i32)
            nc.sync.dma_start(idxp_sb[:], idxp_in[:])
            idx_sb = res.tile([128, NCOL], i32)
            nc.vector.tensor_scalar(out=idx_sb[:], in0=idxp_sb[:],
                                    scalar1=0x1FFFF, scalar2=None,
                                    op0=AL.bitwise_and)
            tmp_i = work.tile([128, NCOL], i32, tag="unp")
            nc.vector.tensor_scalar(out=tmp_i[:], in0=idxp_sb[:],
                                    scalar1=17, scalar2=7,
                                    op0=AL.logical_shift_right,
                                    op1=AL.bitwise_and)
            pos_f = res.tile([128, NCOL], f32)
            nc.vector.tensor_copy(pos_f[:], tmp_i[:])
            tmp_i2 = work.tile([128, NCOL], i32, tag="unp")
            nc.vector.tensor_scalar(out=tmp_i2[:], in0=idxp_sb[:],
                                    scalar1=20, scalar2=None,
                                    op0=AL.logical_shift_right)
            u_f = res.tile([128, NCOL], f32)
            nc.vector.tensor_copy(u_f[:], tmp_i2[:])
            u_bf = res.tile([128, NCOL], bf16)
            nc.vector.tensor_scalar(out=u_bf[:], in0=u_f[:], scalar1=1.0 / UQ,
                                    scalar2=None, op0=AL.mult)

            # ---------------- constants built on device ----------------
            iota7_i = res.tile([128, NPCOL], i32)
            nc.gpsimd.iota(iota7_i[:], pattern=[[1, NPCOL]], base=0,
                           channel_multiplier=0)
            iota7 = res.tile([128, NPCOL], f32)
            nc.vector.tensor_copy(iota7[:], iota7_i[:])
            selfp_i = res.tile([128, NPCOL], i32)
            nc.gpsimd.iota(selfp_i[:], pattern=[[-1, NPCOL]], base=-SLOTS_E,
                           channel_multiplier=1)
            selfpat = res.tile([128, NPCOL], bf16)
            nc.vector.tensor_scalar(out=selfpat[:], in0=selfp_i[:],
                                    scalar1=0, scalar2=None, op0=AL.is_equal)
            gids_i = res.tile([128, N_GRAPHS], i32)
            nc.gpsimd.iota(gids_i[:], pattern=[[1, N_GRAPHS]], base=0,
                           channel_multiplier=0)
            gids_f = res.tile([128, N_GRAPHS], f32)
            nc.vector.tensor_copy(gids_f[:], gids_i[:])

            # ---------------- small inputs ----------------
            deg_sb = res.tile([128, len(CH)], f32)
            nc.sync.dma_start(deg_sb[:], deg_in[:])
            batchv_sb = res.tile([128, len(CH)], f32)
            nc.sync.dma_start(batchv_sb[:], batchv_in[:])
            cntinv_sb = res.tile([128, 4], f32)
            nc.sync.dma_start(cntinv_sb[:], cntinv_in[:])
            fcw_sb = res.tile([F, 8], f32)
            nc.sync.dma_start(fcw_sb[:], fcw_in[:])
            fcb_sb = res.tile([128, 8], f32)
            nc.sync.dma_start(fcb_sb[:], fcb_in[:])
            w_sb = []
            for l in range(4):
                rows = 4 if l == 0 else F + 1
                t = res.tile([rows, 3 * F], f32, tag=f"w{l}")
                nc.sync.dma_start(t[:], wts_in[l][:])
                w_sb.append(t)

            # ---------------- aggregate buffers ----------------
            S_pl = res.tile([F, NLOC], f32)       # sum_j h_j        (transposed)
            S_u = res.tile([F, NLOC], f32)        # sum_j h_j * u    (transposed)
            S_rt = res.tile([F + 1, NLOC], f32)   # h_i (self); row F = ones
            nc.vector.memset(S_rt[F:F + 1, :], 1.0)

            x_tab = nc.dram_tensor("x_tab", [ZROW + 1, 4], bf16,
                                   kind="Internal", addr_space="Shared")
            h_tabs = [
                nc.dram_tensor(f"h_tab{i}", [ZROW + 1, F], bf16,
                               kind="Internal", addr_space="Shared")
                for i in range(2)
            ]
            ag_in = dr.tile([NLOC, F], bf16)
            pool_in = dr.tile([F, N_GRAPHS], f32)
            pool_out = dr.tile([F, N_GRAPHS], f32, addr_space="Shared")

            zrow = res.tile([1, F], bf16)
            nc.vector.memset(zrow[:], 0.0)
            nc.sync.dma_start(x_tab[ZROW:ZROW + 1, :], zrow[:, 0:4])
            for t in h_tabs:
                nc.sync.dma_start(t[ZROW:ZROW + 1, :], zrow[:])

            # gather x across cores (collectives can't read IO tensors
            # directly -> stage through an Internal DRAM buffer)
            xstage = dr.tile([NLOC, 4], bf16)
            nc.sync.dma_start(xstage[:], xloc_in[:])
            nc.gpsimd.collective_compute(
                "AllGather", AL.bypass,
                replica_groups=[list(range(N_CORES))],
                ins=[xstage.opt()],
                outs=[x_tab[0:ZROW, :].opt()],
            )

            pool_ps = ps_p.tile([F, N_GRAPHS], f32, space="PSUM")

            def scat_body(l, fin, table, c0, uu):
                """One bank group of the scatter stage; c0 may be symbolic."""
                idx_st = ibuf.tile([128, BG], i32, tag=f"ist{uu}")
                nc.vector.tensor_copy(idx_st[:], idx_sb[:, ds(c0, BG)])
                g_t = gbuf.tile([128, BG, fin], bf16, tag=f"g{min(l, 1)}_{uu}")
                for c in range(BG):
                    nc.gpsimd.indirect_dma_start(
                        out=g_t[:, c, :], out_offset=None, in_=table,
                        in_offset=bass.IndirectOffsetOnAxis(
                            ap=idx_st[:, c:c + 1], axis=0),
                    )
                p_t = pbuf.tile([128, BG, NPCOL, 3], bf16, tag=f"pat{uu}")
                nc.vector.tensor_tensor(
                    out=p_t[:, :, :, 0],
                    in0=pos_f[:, ds(c0, BG)].unsqueeze(2)
                        .to_broadcast([128, BG, NPCOL]),
                    in1=iota7[:].unsqueeze(1).to_broadcast([128, BG, NPCOL]),
                    op=AL.is_equal)
                nc.vector.tensor_tensor(
                    out=p_t[:, :, :, 1],
                    in0=p_t[:, :, :, 0],
                    in1=u_bf[:, ds(c0, BG)].unsqueeze(2)
                        .to_broadcast([128, BG, NPCOL]),
                    op=AL.mult)
                nc.vector.tensor_copy(
                    p_t[:, :, :, 2],
                    selfpat[:].unsqueeze(1).to_broadcast([128, BG, NPCOL]))
                bank = ps_s.tile([F, BG * PWC], f32, tag=f"scat{uu}",
                                 space="PSUM")
                for c in range(BG):
                    nc.tensor.matmul(
                        bank[0:fin, c * PWC:(c + 1) * PWC],
                        lhsT=g_t[:, c, :],
                        rhs=p_t[:, c].rearrange("p k t -> p (k t)"),
                        start=True, stop=True,
                    )
                bview = bank[0:fin].rearrange("f (c k t) -> f t (c k)",
                                              k=NPCOL, t=3)
                dst = ds(c0 * NPCOL, BG * NPCOL)
                nc.vector.tensor_copy(S_pl[0:fin, dst], bview[:, 0, :])
                nc.vector.tensor_copy(S_u[0:fin, dst], bview[:, 1, :])
                nc.vector.tensor_copy(S_rt[0:fin, dst], bview[:, 2, :])

            def dense_chunk(l, fin, rr, w_t, t0, n, k_idx, uu,
                            symbolic):
                """Dense + deg scale + ELU for nodes [t0, t0+n)."""
                if symbolic:
                    spl = stg.tile([F, 128], f32, tag=f"spl{uu}")
                    nc.vector.tensor_copy(spl[0:fin, 0:n],
                                          S_pl[0:fin, ds(t0, n)])
                    su = stg.tile([F, 128], f32, tag=f"su{uu}")
                    nc.vector.tensor_copy(su[0:fin, 0:n],
                                          S_u[0:fin, ds(t0, n)])
                    srt = stg.tile([F + 1, 128], f32, tag=f"srt{uu}")
                    nc.vector.tensor_copy(srt[0:rr, 0:n],
                                          S_rt[0:rr, ds(t0, n)])
                    spl_ap, su_ap, srt_ap = (spl[0:fin, 0:n], su[0:fin, 0:n],
                                             srt[0:rr, 0:n])
                    degc = deg_sb[0:n, ds(k_idx, 1)]
                else:
                    spl_ap = S_pl[0:fin, t0:t0 + n]
                    su_ap = S_u[0:fin, t0:t0 + n]
                    srt_ap = S_rt[0:rr, t0:t0 + n]
                    degc = deg_sb[0:n, k_idx:k_idx + 1]
                d_ps = ps_d.tile([128, 128], f32, tag=f"dense{uu}",
                                 space="PSUM")
                nc.tensor.matmul(d_ps[0:n, 0:F], lhsT=spl_ap,
                                 rhs=w_t[0:fin, 0:F], start=True, stop=False)
                nc.tensor.matmul(d_ps[0:n, 0:F], lhsT=su_ap,
                                 rhs=w_t[0:fin, F:2 * F], start=False,
                                 stop=True)
                nc.tensor.matmul(d_ps[0:n, F:2 * F], lhsT=srt_ap,
                                 rhs=w_t[0:rr, 2 * F:3 * F], start=True,
                                 stop=True)
                z_t = work.tile([128, F], f32, tag=f"z{uu}")
                nc.vector.tensor_scalar(
                    out=z_t[0:n, :], in0=d_ps[0:n, 0:F],
                    scalar1=degc, scalar2=None, op0=AL.mult)
                nc.vector.tensor_tensor(
                    out=z_t[0:n, :], in0=z_t[0:n, :],
                    in1=d_ps[0:n, F:2 * F], op=AL.add)
                # ELU(z) = max(z, min(exp(z),1) - 1)
                ex_t = work.tile([128, F], f32, tag=f"ex{uu}")
                nc.scalar.activation(ex_t[0:n, :], z_t[0:n, :], ACTF.Exp)
                nc.vector.tensor_scalar(
                    out=ex_t[0:n, :], in0=ex_t[0:n, :],
                    scalar1=1.0, scalar2=-1.0, op0=AL.min, op1=AL.add)
                h_t = work.tile([128, F], bf16 if l < 3 else f32,
                                tag=f"h{uu}_{l < 3}")
                nc.vector.tensor_tensor(
                    out=h_t[0:n, :], in0=z_t[0:n, :], in1=ex_t[0:n, :],
                    op=AL.max)
                return h_t

            NFULL = (NLOC // 128) * 128  # 9984

            for l in range(4):
                fin = 4 if l == 0 else F
                rr = 4 if l == 0 else F + 1  # root matmul contraction rows
                w_t = w_sb[l]
                if l == 0:
                    table = x_tab[:]
                else:
                    table = h_tabs[(l - 1) % 2][:]

                # --- scatter: gather + on-device pattern + matmuls ---
                with tc.For_i(0, NCOL, BG * 2) as i0:
                    for uu in range(2):
                        scat_body(l, fin, table, i0 + uu * BG, uu)

                # --- dense (node-major out) + deg scale + ELU ---
                if l < 3:
                    with tc.For_i(0, NFULL, 256) as i0:
                        for uu in range(2):
                            t0 = i0 + uu * 128
                            h_t = dense_chunk(l, fin, rr, w_t, t0, 128,
                                              t0 // 128, uu, True)
                            nc.sync.dma_start(ag_in[ds(t0, 128), :], h_t[:])
                    # tail chunk
                    n = NLOC - NFULL
                    h_t = dense_chunk(l, fin, rr, w_t, NFULL, n,
                                      NFULL // 128, 0, False)
                    nc.sync.dma_start(ag_in[NFULL:NLOC, :], h_t[0:n, :])
                    nc.gpsimd.collective_compute(
                        "AllGather", AL.bypass,
                        replica_groups=[list(range(N_CORES))],
                        ins=[ag_in.opt()],
                        outs=[h_tabs[l % 2][0:ZROW, :].opt()],
                    )
                else:
                    for k, (t0, t1) in enumerate(CH):
                        n = t1 - t0
                        h_t = dense_chunk(l, fin, rr, w_t, t0, n, k,
                                          k % 2, False)
                        if n < 128:
                            nc.vector.memset(h_t[n:128, :], 0.0)
                        oh_t = work.tile([128, N_GRAPHS], f32, tag="oh")
                        nc.vector.tensor_scalar(
                            out=oh_t[:], in0=gids_f[:],
                            scalar1=batchv_sb[:, k:k + 1], scalar2=None,
                            op0=AL.is_equal)
                        nc.tensor.matmul(
                            pool_ps[:], lhsT=h_t[:], rhs=oh_t[:],
                            start=(k == 0), stop=(k == len(CH) - 1))

            # ---------------- pooling all-reduce + head ----------------
            pool_sb = res.tile([F, N_GRAPHS], f32)
            nc.vector.tensor_copy(pool_sb[:], pool_ps[:])
            nc.sync.dma_start(pool_in[:], pool_sb[:])
            nc.gpsimd.collective_compute(
                "AllReduce", AL.add,
                replica_groups=[list(range(N_CORES))],
                ins=[pool_in.opt()], outs=[pool_out.opt()],
            )
            pooled = res.tile([F, N_GRAPHS], f32)
            nc.sync.dma_start(pooled[:], pool_out[:])

            for gch in range(N_GRAPHS // 128):
                g0 = gch * 128
                l_ps = ps_p.tile([128, 8], f32, tag="head", space="PSUM")
                nc.tensor.matmul(
                    l_ps[:, 0:8], lhsT=pooled[:, g0:g0 + 128], rhs=fcw_sb[:],
                    start=True, stop=True)
                z_t = work.tile([128, 8], f32, tag="hz")
                nc.vector.tensor_scalar(
                    out=z_t[:], in0=l_ps[:],
                    scalar1=cntinv_sb[:, gch:gch + 1], scalar2=None,
                    op0=AL.mult)
                nc.vector.tensor_tensor(out=z_t[:], in0=z_t[:], in1=fcb_sb[:],
                                        op=AL.add)
                rm = work.tile([128, 1], f32, tag="rm")
                nc.vector.tensor_reduce(rm[:], z_t[:, 0:6], axis=AX.X, op=AL.max)
                zs = work.tile([128, 8], f32, tag="zs")
                nc.vector.tensor_scalar(
                    out=zs[:], in0=z_t[:], scalar1=rm[:], scalar2=None,
                    op0=AL.subtract)
                e_t = work.tile([128, 8], f32, tag="et")
                nc.scalar.activation(e_t[:, 0:6], zs[:, 0:6], ACTF.Exp)
                sm = work.tile([128, 1], f32, tag="sm")
                nc.vector.tensor_reduce(sm[:], e_t[:, 0:6], axis=AX.X, op=AL.add)
                ln = work.tile([128, 1], f32, tag="ln")
                nc.scalar.activation(ln[:], sm[:], ACTF.Ln)
                oT = work.tile([128, 8], f32, tag="oT")
                nc.vector.tensor_scalar(
                    out=oT[:], in0=zs[:], scalar1=ln[:], scalar2=None,
                    op0=AL.subtract)
                nc.sync.dma_start(out_logits[g0:g0 + 128, :], oT[:])

    nc.compile()
    return nc


def make_in_maps(plan, x, weights):
    import ml_dtypes
    x = np.asarray(x, dtype=np.float32)
    perm_row, loc_row, core_of = plan["perm_row"], plan["loc_row"], plan["core_of"]

    xloc = np.zeros((N_CORES, NLOC, 4), dtype=np.float32)
    xloc[core_of, loc_row, 0:3] = x
    xloc[core_of, loc_row, 3] = 1.0
    xloc = xloc.astype(ml_dtypes.bfloat16)

    fcb = np.zeros((128, 8), dtype=np.float32)
    fcb[:, :6] = np.asarray(weights["fc_b"], dtype=np.float32)
    fcw = np.zeros((F, 8), dtype=np.float32)
    fcw[:, :6] = np.asarray(weights["fc_w"], dtype=np.float32)
    cntinv = np.ascontiguousarray(
        plan["cnt_inv"].reshape(4, 128).T).astype(np.float32)

    wps = []
    for l in range(4):
        W = np.asarray(weights[f"W{l+1}"], dtype=np.float32)
        root = np.asarray(weights[f"root{l+1}"], dtype=np.float32)
        b = np.asarray(weights[f"b{l+1}"], dtype=np.float32)
        rows = 4 if l == 0 else F + 1
        wp = np.zeros((rows, 3 * F), np.float32)
        fin_d = W.shape[1]  # 3 or 64
        wp[:fin_d, 0:F] = W[0]
        wp[:fin_d, F:2 * F] = W[1] - W[0]
        wp[:fin_d, 2 * F:3 * F] = root
        wp[rows - 1, 2 * F:3 * F] = b  # bias rides the ones row
        wps.append(wp)

    in_maps = []
    for c in range(N_CORES):
        im = {
            "idxp": plan["idxp"][c],
            "xloc": xloc[c],
            "degnm": plan["deg_nm"][c],
            "batchv": plan["batch_nm"][c],
            "cntinv": cntinv,
            "fcw": fcw, "fcb": fcb,
        }
        for l in range(4):
            im[f"wpack_{l}"] = wps[l]
        in_maps.append(im)
    return in_maps


_NC_CACHE = {}


def kernel(**inputs):
    x = np.asarray(inputs["x"], dtype=np.float32)
    pseudo = np.asarray(inputs["pseudo"], dtype=np.float32)
    edge_index = np.asarray(inputs["edge_index"]).astype(np.int64)
    batch = np.asarray(inputs["batch"]).astype(np.int64)
    weights = {k: np.asarray(inputs[k], dtype=np.float32) for k in
               ["W1", "root1", "b1", "W2", "root2", "b2", "W3", "root3",
                "b3", "W4", "root4", "b4", "fc_w", "fc_b"]}

    plan = build_plan(edge_index, pseudo, batch)
    in_maps = make_in_maps(plan, x, weights)

    if "nc" not in _NC_CACHE:
        _NC_CACHE["nc"] = build_nc()
    nc = _NC_CACHE["nc"]

    res = run_bass_kernel_spmd(nc, in_maps, core_ids=list(range(N_CORES)))
    return np.ascontiguousarray(res.results[0]["out_logits"][:, :6]).astype(np.float32)


# revision 25
# speedup vs baseline: 19.0288x; 1.0407x over previous
"""Self-contained Trainium2 Bass kernel for the 4-layer SplineConv GNN.

kernel(**inputs) takes the FULL unsharded inputs (x, pseudo, edge_index,
batch, W1..W4, root1..4, b1..4, fc_w, fc_b) and returns log_softmax logits
[512, 6] float32, computed on 8 NeuronCores.

Sharding: nodes/edges partitioned by dst range across cores; per-core
column packing (7 nodes x 128 slots per PE column); per-layer AllGather of
node features; AllReduce of pooled per-graph sums.

Upload-minimized: per-edge data is packed into ONE int32 per slot
(17-bit row index | 3-bit in-column position | 12-bit quantized u) and the
spline pattern matrices are reconstructed on-device. The root/bias terms
ride along as "self edges" in reserved slots 121..127, which also lets the
dense matmul emit node-major output directly (no transpose stage).
"""
import numpy as np
import jax

# Persistent executable cache: run_bass_kernel_spmd re-jits per call; without
# this every call re-runs the walrus NEFF packager (~2s). With it, warm calls
# fetch the compiled executable from disk.
jax.config.update("jax_compilation_cache_dir", "/tmp/jax_cc_cache")
jax.config.update("jax_persistent_cache_min_entry_size_bytes", -1)
jax.config.update("jax_persistent_cache_min_compile_time_secs", 0.0)

import concourse.bass as bass
import concourse.bacc as bacc
import concourse.mybir as mybir
import concourse.tile as tile
from concourse.bass import ds
from concourse.bass_utils import run_bass_kernel_spmd


N_CORES = 8
N_NODES = 80000
N_GRAPHS = 512
NPC = N_NODES // N_CORES     # nodes per core (10000)
NPCOL = 7                    # nodes per column
SLOTS_E = 121                # edge slots per column (121..127 are self slots)
NCOL = 1440                  # columns per core
BG = 24                      # columns per PSUM bank group (24*21=504<=512)
NBG = NCOL // BG             # 60
PWC = 3 * NPCOL              # pattern cols per column (mask, mask*u, self)
NLOC = NCOL * NPCOL          # local node slots per core (10080)
ZROW = N_CORES * NLOC        # zero row index in tables (80640)
F = 64
UQ = 4096.0                  # 12-bit u quantization

f32 = mybir.dt.float32
bf16 = mybir.dt.bfloat16
i32 = mybir.dt.int32
AL = mybir.AluOpType
ACTF = mybir.ActivationFunctionType
AX = mybir.AxisListType

CH = [(i * 128, min((i + 1) * 128, NLOC)) for i in range((NLOC + 127) // 128)]


def build_plan(edge_index, pseudo, batch):
    src = np.asarray(edge_index[0], dtype=np.int64)
    dst = np.asarray(edge_index[1], dtype=np.int64)
    u = np.asarray(pseudo, dtype=np.float32).reshape(-1)
    batch = np.asarray(batch, dtype=np.int64)
    E = src.shape[0]

    deg = np.bincount(dst, minlength=N_NODES).astype(np.int64)
    deg_clip = np.maximum(deg, 1).astype(np.float32)

    # sort edges by dst for per-node grouping
    order = np.argsort(dst, kind="stable")
    s_src, s_dst, s_u = src[order], dst[order], u[order]
    rowptr = np.zeros(N_NODES + 1, dtype=np.int64)
    np.cumsum(deg, out=rowptr[1:])

    # --- per-core column packing: LPT bin packing, capacity 7 nodes/col ---
    import heapq
    col_of = np.empty(N_NODES, dtype=np.int64)
    pos_of = np.empty(N_NODES, dtype=np.int64)
    for c in range(N_CORES):
        nodes = np.arange(c * NPC, (c + 1) * NPC)
        sorted_nodes = nodes[np.argsort(-deg[nodes], kind="stable")]
        heap = [(0, j) for j in range(NCOL)]  # (load, col); cols start empty
        counts = np.zeros(NCOL, dtype=np.int64)
        loads = np.zeros(NCOL, dtype=np.int64)
        spill = []
        degs = deg[sorted_nodes]
        for g, d in zip(sorted_nodes.tolist(), degs.tolist()):
            while True:
                load, j = heapq.heappop(heap)
                if counts[j] < NPCOL:
                    break
            col_of[g] = j
            pos_of[g] = counts[j]
            counts[j] += 1
            loads[j] = load + d
            if counts[j] < NPCOL:
                heapq.heappush(heap, (load + d, j))
        assert loads.max() <= SLOTS_E, f"col overload {loads.max()}"

    core_of = np.arange(N_NODES) // NPC
    perm_row = core_of * NLOC + col_of * NPCOL + pos_of  # global node -> table row

    # --- packed slot table: row | pos<<17 | qu<<20 ---
    EMPTY = np.uint32(ZROW | (7 << 17))
    idxp = np.full((N_CORES, 128, NCOL), EMPTY, dtype=np.uint32)

    # edge slots: per (core,col), nodes at pos 0..6 occupy consecutive slots
    deg_cp = np.zeros((N_CORES, NCOL, NPCOL), dtype=np.int64)
    deg_cp[core_of, col_of, pos_of] = deg
    start_cp = np.cumsum(deg_cp, axis=2) - deg_cp  # exclusive cumsum over pos
    slot_start = start_cp[core_of, col_of, pos_of]  # per node

    e_idx = np.arange(E, dtype=np.int64)
    within = e_idx - rowptr[s_dst]
    e_slot = slot_start[s_dst] + within
    e_core = core_of[s_dst]
    e_col = col_of[s_dst]
    qu = np.minimum(np.rint(s_u * UQ), UQ - 1).astype(np.uint32)
    packed = perm_row[s_src].astype(np.uint32) \
        | (pos_of[s_dst].astype(np.uint32) << 17) | (qu << 20)
    idxp[e_core, e_slot, e_col] = packed

    # self slots: slot 121+p gathers node's own row (pos=7, u=0 -> only the
    # constant self pattern column reads it)
    idxp[core_of, SLOTS_E + pos_of, col_of] = \
        perm_row.astype(np.uint32) | np.uint32(7 << 17)

    # --- per-node metadata in node-major chunk layout [128, n_chunks] ---
    nch = len(CH)
    deg_nm = np.zeros((N_CORES, 128 * nch), dtype=np.float32)
    batch_nm = np.full((N_CORES, 128 * nch), float(N_GRAPHS), dtype=np.float32)
    loc_row = col_of * NPCOL + pos_of
    deg_nm[core_of, loc_row] = 1.0 / deg_clip
    batch_nm[core_of, loc_row] = batch.astype(np.float32)
    deg_nm = deg_nm.reshape(N_CORES, nch, 128).transpose(0, 2, 1)
    batch_nm = batch_nm.reshape(N_CORES, nch, 128).transpose(0, 2, 1)

    # --- x table rows in local order, 4th channel = 1 (bias carrier) ---
    cnt = np.bincount(batch, minlength=N_GRAPHS).astype(np.float32)
    cnt_inv = (1.0 / np.maximum(cnt, 1.0)).astype(np.float32)

    return dict(idxp=idxp.view(np.int32), perm_row=perm_row,
                deg_nm=np.ascontiguousarray(deg_nm),
                batch_nm=np.ascontiguousarray(batch_nm),
                cnt_inv=cnt_inv, loc_row=loc_row, core_of=core_of)


def build_nc():
    nc = bacc.Bacc("TRN2", target_bir_lowering=False)

    idxp_in = nc.dram_tensor("idxp", [128, NCOL], i32, kind="ExternalInput")
    xloc_in = nc.dram_tensor("xloc", [NLOC, 4], bf16, kind="ExternalInput")
    deg_in = nc.dram_tensor("degnm", [128, len(CH)], f32, kind="ExternalInput")
    batchv_in = nc.dram_tensor("batchv", [128, len(CH)], f32, kind="ExternalInput")
    cntinv_in = nc.dram_tensor("cntinv", [128, 4], f32, kind="ExternalInput")
    fcw_in = nc.dram_tensor("fcw", [F, 8], f32, kind="ExternalInput")
    fcb_in = nc.dram_tensor("fcb", [128, 8], f32, kind="ExternalInput")
    wts_in = []
    for l in range(4):
        rows = 4 if l == 0 else F + 1
        wts_in.append(nc.dram_tensor(f"wpack_{l}", [rows, 3 * F], f32,
                                     kind="ExternalInput"))

    out_logits = nc.dram_tensor("out_logits", [N_GRAPHS, 8], f32,
                                kind="ExternalOutput")

    with tile.TileContext(nc) as tc:
        with (
            tc.tile_pool(name="res", bufs=1) as res,
            tc.tile_pool(name="gbuf", bufs=1) as gbuf,
            tc.tile_pool(name="pbuf", bufs=1) as pbuf,
            tc.tile_pool(name="ibuf", bufs=1) as ibuf,
            tc.tile_pool(name="sbuf_st", bufs=1) as stg,
            tc.tile_pool(name="work", bufs=2) as work,
            tc.tile_pool(name="psum_s", bufs=1, space="PSUM") as ps_s,
            tc.tile_pool(name="psum_d", bufs=1, space="PSUM") as ps_d,
            tc.tile_pool(name="psum_p", bufs=1, space="PSUM") as ps_p,
            tc.tile_pool(name="dram", bufs=1, space="DRAM") as dr,
        ):
            # ---------------- unpack slot table ----------------
            idxp_sb = res.tile([128, NCOL], i32)
            nc.sync.dma_start(idxp_sb[:], idxp_in[:])
            idx_sb = res.tile([128, NCOL], i32)
            nc.vector.tensor_scalar(out=idx_sb[:], in0=idxp_sb[:],
                                    scalar1=0x1FFFF, scalar2=None,
                                    op0=AL.bitwise_and)
            tmp_i = work.tile([128, NCOL], i32, tag="unp")
            nc.vector.tensor_scalar(out=tmp_i[:], in0=idxp_sb[:],
                                    scalar1=17, scalar2=7,
                                    op0=AL.logical_shift_right,
                                    op1=AL.bitwise_and)
            pos_f = res.tile([128, NCOL], f32)
            nc.vector.tensor_copy(pos_f[:], tmp_i[:])
            tmp_i2 = work.tile([128, NCOL], i32, tag="unp")
            nc.vector.tensor_scalar(out=tmp_i2[:], in0=idxp_sb[:],
                                    scalar1=20, scalar2=None,
                                    op0=AL.logical_shift_right)
            u_f = res.tile([128, NCOL], f32)
            nc.vector.tensor_copy(u_f[:], tmp_i2[:])
            u_bf = res.tile([128, NCOL], bf16)
            nc.vector.tensor_scalar(out=u_bf[:], in0=u_f[:], scalar1=1.0 / UQ,
                                    scalar2=None, op0=AL.mult)

            # ---------------- constants built on device ----------------
            iota7_i = res.tile([128, NPCOL], i32)
            nc.gpsimd.iota(iota7_i[:], pattern=[[1, NPCOL]], base=0,
                           channel_multiplier=0)
            iota7 = res.tile([128, NPCOL], f32)
            nc.vector.tensor_copy(iota7[:], iota7_i[:])
            selfp_i = res.tile([128, NPCOL], i32)
            nc.gpsimd.iota(selfp_i[:], pattern=[[-1, NPCOL]], base=-SLOTS_E,
                           channel_multiplier=1)
            selfpat = res.tile([128, NPCOL], bf16)
            nc.vector.tensor_scalar(out=selfpat[:], in0=selfp_i[:],
                                    scalar1=0, scalar2=None, op0=AL.is_equal)
            gids_i = res.tile([128, N_GRAPHS], i32)
            nc.gpsimd.iota(gids_i[:], pattern=[[1, N_GRAPHS]], base=0,
                           channel_multiplier=0)
            gids_f = res.tile([128, N_GRAPHS], f32)
            nc.vector.tensor_copy(gids_f[:], gids_i[:])

            # ---------------- small inputs ----------------
            deg_sb = res.tile([128, len(CH)], f32)
            nc.sync.dma_start(deg_sb[:], deg_in[:])
            batchv_sb = res.tile([128, len(CH)], f32)
            nc.sync.dma_start(batchv_sb[:], batchv_in[:])
            cntinv_sb = res.tile([128, 4], f32)
            nc.sync.dma_start(cntinv_sb[:], cntinv_in[:])
            fcw_sb = res.tile([F, 8], f32)
            nc.sync.dma_start(fcw_sb[:], fcw_in[:])
            fcb_sb = res.tile([128, 8], f32)
            nc.sync.dma_start(fcb_sb[:], fcb_in[:])
            w_sb = []
            for l in range(4):
                rows = 4 if l == 0 else F + 1
                t = res.tile([rows, 3 * F], f32, tag=f"w{l}")
                nc.sync.dma_start(t[:], wts_in[l][:])
                w_sb.append(t)

            # ---------------- aggregate buffers ----------------
            S_pl = res.tile([F, NLOC], f32)       # sum_j h_j        (transposed)
            S_u = res.tile([F, NLOC], f32)        # sum_j h_j * u    (transposed)
            S_rt = res.tile([F + 1, NLOC], f32)   # h_i (self); row F = ones
            nc.vector.memset(S_rt[F:F + 1, :], 1.0)

            x_tab = nc.dram_tensor("x_tab", [ZROW + 1, 4], bf16,
                                   kind="Internal", addr_space="Shared")
            h_tabs = [
                nc.dram_tensor(f"h_tab{i}", [ZROW + 1, F], bf16,
                               kind="Internal", addr_space="Shared")
                for i in range(2)
            ]
            ag_in = dr.tile([NLOC, F], bf16)
            pool_in = dr.tile([F, N_GRAPHS], f32)
            pool_out = dr.tile([F, N_GRAPHS], f32, addr_space="Shared")

            zrow = res.tile([1, F], bf16)
            nc.vector.memset(zrow[:], 0.0)
            nc.sync.dma_start(x_tab[ZROW:ZROW + 1, :], zrow[:, 0:4])
            for t in h_tabs:
                nc.sync.dma_start(t[ZROW:ZROW + 1, :], zrow[:])

            # gather x across cores (collectives can't read IO tensors
            # directly -> stage through an Internal DRAM buffer)
            xstage = dr.tile([NLOC, 4], bf16)
            nc.sync.dma_start(xstage[:], xloc_in[:])
            nc.gpsimd.collective_compute(
                "AllGather", AL.bypass,
                replica_groups=[list(range(N_CORES))],
                ins=[xstage.opt()],
                outs=[x_tab[0:ZROW, :].opt()],
            )

            pool_ps = ps_p.tile([F, N_GRAPHS], f32, space="PSUM")

            def scat_body(l, fin, table, c0, uu):
                """One bank group of the scatter stage; c0 may be symbolic."""
                idx_st = ibuf.tile([128, BG], i32, tag=f"ist{uu}")
                nc.vector.tensor_copy(idx_st[:], idx_sb[:, ds(c0, BG)])
                g_t = gbuf.tile([128, BG, fin], bf16, tag=f"g{min(l, 1)}_{uu}")
                for c in range(BG):
                    nc.gpsimd.indirect_dma_start(
                        out=g_t[:, c, :], out_offset=None, in_=table,
                        in_offset=bass.IndirectOffsetOnAxis(
                            ap=idx_st[:, c:c + 1], axis=0),
                    )
                p_t = pbuf.tile([128, BG, NPCOL, 3], bf16, tag=f"pat{uu}")
                nc.vector.tensor_tensor(
                    out=p_t[:, :, :, 0],
                    in0=pos_f[:, ds(c0, BG)].unsqueeze(2)
                        .to_broadcast([128, BG, NPCOL]),
                    in1=iota7[:].unsqueeze(1).to_broadcast([128, BG, NPCOL]),
                    op=AL.is_equal)
                nc.vector.tensor_tensor(
                    out=p_t[:, :, :, 1],
                    in0=p_t[:, :, :, 0],
                    in1=u_bf[:, ds(c0, BG)].unsqueeze(2)
                        .to_broadcast([128, BG, NPCOL]),
                    op=AL.mult)
                nc.vector.tensor_copy(
                    p_t[:, :, :, 2],
                    selfpat[:].unsqueeze(1).to_broadcast([128, BG, NPCOL]))
                bank = ps_s.tile([F, BG * PWC], f32, tag=f"scat{uu}",
                                 space="PSUM")
                for c in range(BG):
                    nc.tensor.matmul(
                        bank[0:fin, c * PWC:(c + 1) * PWC],
                        lhsT=g_t[:, c, :],
                        rhs=p_t[:, c].rearrange("p k t -> p (k t)"),
                        start=True, stop=True,
                    )
                bview = bank[0:fin].rearrange("f (c k t) -> f t (c k)",
                                              k=NPCOL, t=3)
                dst = ds(c0 * NPCOL, BG * NPCOL)
                nc.vector.tensor_copy(S_pl[0:fin, dst], bview[:, 0, :])
                nc.vector.tensor_copy(S_u[0:fin, dst], bview[:, 1, :])
                nc.vector.tensor_copy(S_rt[0:fin, dst], bview[:, 2, :])

            def dense_chunk(l, fin, rr, w_t, t0, n, k_idx, uu,
                            symbolic):
                """Dense + deg scale + ELU for nodes [t0, t0+n)."""
                if symbolic:
                    spl = stg.tile([F, 128], f32, tag=f"spl{uu}")
                    nc.vector.tensor_copy(spl[0:fin, 0:n],
                                          S_pl[0:fin, ds(t0, n)])
                    su = stg.tile([F, 128], f32, tag=f"su{uu}")
                    nc.vector.tensor_copy(su[0:fin, 0:n],
                                          S_u[0:fin, ds(t0, n)])
                    srt = stg.tile([F + 1, 128], f32, tag=f"srt{uu}")
                    nc.vector.tensor_copy(srt[0:rr, 0:n],
                                          S_rt[0:rr, ds(t0, n)])
                    spl_ap, su_ap, srt_ap = (spl[0:fin, 0:n], su[0:fin, 0:n],
                                             srt[0:rr, 0:n])
                    degc = deg_sb[0:n, ds(k_idx, 1)]
                else:
                    spl_ap = S_pl[0:fin, t0:t0 + n]
                    su_ap = S_u[0:fin, t0:t0 + n]
                    srt_ap = S_rt[0:rr, t0:t0 + n]
                    degc = deg_sb[0:n, k_idx:k_idx + 1]
                d_ps = ps_d.tile([128, 128], f32, tag=f"dense{uu}",
                                 space="PSUM")
                nc.tensor.matmul(d_ps[0:n, 0:F], lhsT=spl_ap,
                                 rhs=w_t[0:fin, 0:F], start=True, stop=False)
                nc.tensor.matmul(d_ps[0:n, 0:F], lhsT=su_ap,
                                 rhs=w_t[0:fin, F:2 * F], start=False,
                                 stop=True)
                nc.tensor.matmul(d_ps[0:n, F:2 * F], lhsT=srt_ap,
                                 rhs=w_t[0:rr, 2 * F:3 * F], start=True,
                                 stop=True)
                z_t = work.tile([128, F], f32, tag=f"z{uu}")
                nc.vector.tensor_scalar(
                    out=z_t[0:n, :], in0=d_ps[0:n, 0:F],
                    scalar1=degc, scalar2=None, op0=AL.mult)
                nc.vector.tensor_tensor(
                    out=z_t[0:n, :], in0=z_t[0:n, :],
                    in1=d_ps[0:n, F:2 * F], op=AL.add)
                # ELU(z) = max(z, min(exp(z),1) - 1)
                ex_t = work.tile([128, F], f32, tag=f"ex{uu}")
                nc.scalar.activation(ex_t[0:n, :], z_t[0:n, :], ACTF.Exp)
                nc.vector.tensor_scalar(
                    out=ex_t[0:n, :], in0=ex_t[0:n, :],
                    scalar1=1.0, scalar2=-1.0, op0=AL.min, op1=AL.add)
                h_t = work.tile([128, F], bf16 if l < 3 else f32,
                                tag=f"h{uu}_{l < 3}")
                nc.vector.tensor_tensor(
                    out=h_t[0:n, :], in0=z_t[0:n, :], in1=ex_t[0:n, :],
                    op=AL.max)
                return h_t

            NFULL = (NLOC // 128) * 128  # 9984

            for l in range(4):
                fin = 4 if l == 0 else F
                rr = 4 if l == 0 else F + 1  # root matmul contraction rows
                w_t = w_sb[l]
                if l == 0:
                    table = x_tab[:]
                else:
                    table = h_tabs[(l - 1) % 2][:]

                # --- scatter: gather + on-device pattern + matmuls ---
                with tc.For_i(0, NCOL, BG * 3, staggered_reset=True) as i0:
                    for uu in range(3):
                        scat_body(l, fin, table, i0 + uu * BG, uu)

                # --- dense (node-major out) + deg scale + ELU ---
                if l < 3:
                    with tc.For_i(0, NFULL, 256, staggered_reset=True) as i0:
                        for uu in range(2):
                            t0 = i0 + uu * 128
                            h_t = dense_chunk(l, fin, rr, w_t, t0, 128,
                                              t0 // 128, uu, True)
                            nc.sync.dma_start(ag_in[ds(t0, 128), :], h_t[:])
                    # tail chunk
                    n = NLOC - NFULL
                    h_t = dense_chunk(l, fin, rr, w_t, NFULL, n,
                                      NFULL // 128, 0, False)
                    nc.sync.dma_start(ag_in[NFULL:NLOC, :], h_t[0:n, :])
                    nc.gpsimd.collective_compute(
                        "AllGather", AL.bypass,
                        replica_groups=[list(range(N_CORES))],
                        ins=[ag_in.opt()],
                        outs=[h_tabs[l % 2][0:ZROW, :].opt()],
                    )
                else:
                    for k, (t0, t1) in enumerate(CH):
                        n = t1 - t0
                        h_t = dense_chunk(l, fin, rr, w_t, t0, n, k,
                                          k % 2, False)
                        if n < 128:
                            nc.vector.memset(h_t[n:128, :], 0.0)
                        oh_t = work.tile([128, N_GRAPHS], f32, tag="oh")
                        nc.vector.tensor_scalar(
                            out=oh_t[:], in0=gids_f[:],
                            scalar1=batchv_sb[:, k:k + 1], scalar2=None,
                            op0=AL.is_equal)
                        nc.tensor.matmul(
                            pool_ps[:], lhsT=h_t[:], rhs=oh_t[:],
                            start=(k == 0), stop=(k == len(CH) - 1))

            # ---------------- pooling all-reduce + head ----------------
            pool_sb = res.tile([F, N_GRAPHS], f32)
            nc.vector.tensor_copy(pool_sb[:], pool_ps[:])
            nc.sync.dma_start(pool_in[:], pool_sb[:])
            nc.gpsimd.collective_compute(
                "AllReduce", AL.add,
                replica_groups=[list(range(N_CORES))],
                ins=[pool_in.opt()], outs=[pool_out.opt()],
            )
            pooled = res.tile([F, N_GRAPHS], f32)
            nc.sync.dma_start(pooled[:], pool_out[:])

            for gch in range(N_GRAPHS // 128):
                g0 = gch * 128
                l_ps = ps_p.tile([128, 8], f32, tag="head", space="PSUM")
                nc.tensor.matmul(
                    l_ps[:, 0:8], lhsT=pooled[:, g0:g0 + 128], rhs=fcw_sb[:],
                    start=True, stop=True)
                z_t = work.tile([128, 8], f32, tag="hz")
                nc.vector.tensor_scalar(
                    out=z_t[:], in0=l_ps[:],
                    scalar1=cntinv_sb[:, gch:gch + 1], scalar2=None,
                    op0=AL.mult)
                nc.vector.tensor_tensor(out=z_t[:], in0=z_t[:], in1=fcb_sb[:],
                                        op=AL.add)
                rm = work.tile([128, 1], f32, tag="rm")
                nc.vector.tensor_reduce(rm[:], z_t[:, 0:6], axis=AX.X, op=AL.max)
                zs = work.tile([128, 8], f32, tag="zs")
                nc.vector.tensor_scalar(
                    out=zs[:], in0=z_t[:], scalar1=rm[:], scalar2=None,
                    op0=AL.subtract)
                e_t = work.tile([128, 8], f32, tag="et")
                nc.scalar.activation(e_t[:, 0:6], zs[:, 0:6], ACTF.Exp)
                sm = work.tile([128, 1], f32, tag="sm")
                nc.vector.tensor_reduce(sm[:], e_t[:, 0:6], axis=AX.X, op=AL.add)
                ln = work.tile([128, 1], f32, tag="ln")
                nc.scalar.activation(ln[:], sm[:], ACTF.Ln)
                oT = work.tile([128, 8], f32, tag="oT")
                nc.vector.tensor_scalar(
                    out=oT[:], in0=zs[:], scalar1=ln[:], scalar2=None,
                    op0=AL.subtract)
                nc.sync.dma_start(out_logits[g0:g0 + 128, :], oT[:])

    nc.compile()
    return nc


def make_in_maps(plan, x, weights):
    import ml_dtypes
    x = np.asarray(x, dtype=np.float32)
    perm_row, loc_row, core_of = plan["perm_row"], plan["loc_row"], plan["core_of"]

    xloc = np.zeros((N_CORES, NLOC, 4), dtype=np.float32)
    xloc[core_of, loc_row, 0:3] = x
    xloc[core_of, loc_row, 3] = 1.0
    xloc = xloc.astype(ml_dtypes.bfloat16)

    fcb = np.zeros((128, 8), dtype=np.float32)
    fcb[:, :6] = np.asarray(weights["fc_b"], dtype=np.float32)
    fcw = np.zeros((F, 8), dtype=np.float32)
    fcw[:, :6] = np.asarray(weights["fc_w"], dtype=np.float32)
    cntinv = np.ascontiguousarray(
        plan["cnt_inv"].reshape(4, 128).T).astype(np.float32)

    wps = []
    for l in range(4):
        W = np.asarray(weights[f"W{l+1}"], dtype=np.float32)
        root = np.asarray(weights[f"root{l+1}"], dtype=np.float32)
        b = np.asarray(weights[f"b{l+1}"], dtype=np.float32)
        rows = 4 if l == 0 else F + 1
        wp = np.zeros((rows, 3 * F), np.float32)
        fin_d = W.shape[1]  # 3 or 64
        wp[:fin_d, 0:F] = W[0]
        wp[:fin_d, F:2 * F] = W[1] - W[0]
        wp[:fin_d, 2 * F:3 * F] = root
        wp[rows - 1, 2 * F:3 * F] = b  # bias rides the ones row
        wps.append(wp)

    in_maps = []
    for c in range(N_CORES):
        im = {
            "idxp": plan["idxp"][c],
            "xloc": xloc[c],
            "degnm": plan["deg_nm"][c],
            "batchv": plan["batch_nm"][c],
            "cntinv": cntinv,
            "fcw": fcw, "fcb": fcb,
        }
        for l in range(4):
            im[f"wpack_{l}"] = wps[l]
        in_maps.append(im)
    return in_maps


_NC_CACHE = {}


def kernel(**inputs):
    x = np.asarray(inputs["x"], dtype=np.float32)
    pseudo = np.asarray(inputs["pseudo"], dtype=np.float32)
    edge_index = np.asarray(inputs["edge_index"]).astype(np.int64)
    batch = np.asarray(inputs["batch"]).astype(np.int64)
    weights = {k: np.asarray(inputs[k], dtype=np.float32) for k in
               ["W1", "root1", "b1", "W2", "root2", "b2", "W3", "root3",
                "b3", "W4", "root4", "b4", "fc_w", "fc_b"]}

    plan = build_plan(edge_index, pseudo, batch)
    in_maps = make_in_maps(plan, x, weights)

    if "nc" not in _NC_CACHE:
        _NC_CACHE["nc"] = build_nc()
    nc = _NC_CACHE["nc"]

    res = run_bass_kernel_spmd(nc, in_maps, core_ids=list(range(N_CORES)))
    return np.ascontiguousarray(res.results[0]["out_logits"][:, :6]).astype(np.float32)


# revision 27
# speedup vs baseline: 20.2116x; 1.0622x over previous
"""Self-contained Trainium2 Bass kernel for the 4-layer SplineConv GNN.

kernel(**inputs) takes the FULL unsharded inputs (x, pseudo, edge_index,
batch, W1..W4, root1..4, b1..4, fc_w, fc_b) and returns log_softmax logits
[512, 6] float32, computed on 8 NeuronCores.

Sharding: nodes/edges partitioned by dst range across cores; per-core
column packing (7 nodes x 128 slots per PE column); per-layer AllGather of
node features; AllReduce of pooled per-graph sums.

Upload-minimized: per-edge data is packed into ONE int32 per slot
(17-bit row index | 3-bit in-column position | 12-bit quantized u) and the
spline pattern matrices are reconstructed on-device. The root/bias terms
ride along as "self edges" in reserved slots 121..127, which also lets the
dense matmul emit node-major output directly (no transpose stage).
"""
import numpy as np
import jax

# Persistent executable cache: run_bass_kernel_spmd re-jits per call; without
# this every call re-runs the walrus NEFF packager (~2s). With it, warm calls
# fetch the compiled executable from disk.
try:
    jax.config.update("jax_compilation_cache_dir", "/tmp/jax_cc_cache")
    jax.config.update("jax_persistent_cache_min_entry_size_bytes", -1)
    jax.config.update("jax_persistent_cache_min_compile_time_secs", 0.0)
except Exception:
    pass

import concourse.bass as bass
import concourse.bacc as bacc
import concourse.mybir as mybir
import concourse.tile as tile
from concourse.bass import ds
from concourse.bass_utils import run_bass_kernel_spmd


N_CORES = 8
N_NODES = 80000
N_GRAPHS = 512
NPC = N_NODES // N_CORES     # nodes per core (10000)
NPCOL = 7                    # nodes per column
SLOTS_E = 121                # edge slots per column (121..127 are self slots)
NCOL = 1440                  # columns per core
BG = 24                      # columns per PSUM bank group (24*21=504<=512)
NBG = NCOL // BG             # 60
PWC = 3 * NPCOL              # pattern cols per column (mask, mask*u, self)
NLOC = NCOL * NPCOL          # local node slots per core (10080)
ZROW = N_CORES * NLOC        # zero row index in tables (80640)
F = 64
UQ = 4096.0                  # 12-bit u quantization

f32 = mybir.dt.float32
bf16 = mybir.dt.bfloat16
i32 = mybir.dt.int32
AL = mybir.AluOpType
ACTF = mybir.ActivationFunctionType
AX = mybir.AxisListType

CH = [(i * 128, min((i + 1) * 128, NLOC)) for i in range((NLOC + 127) // 128)]


def build_plan(edge_index, pseudo, batch):
    src = np.asarray(edge_index[0], dtype=np.int64)
    dst = np.asarray(edge_index[1], dtype=np.int64)
    u = np.asarray(pseudo, dtype=np.float32).reshape(-1)
    batch = np.asarray(batch, dtype=np.int64)
    E = src.shape[0]

    deg = np.bincount(dst, minlength=N_NODES).astype(np.int64)
    deg_clip = np.maximum(deg, 1).astype(np.float32)

    # sort edges by dst for per-node grouping
    order = np.argsort(dst, kind="stable")
    s_src, s_dst, s_u = src[order], dst[order], u[order]
    rowptr = np.zeros(N_NODES + 1, dtype=np.int64)
    np.cumsum(deg, out=rowptr[1:])

    # --- per-core column packing: LPT bin packing, capacity 7 nodes/col ---
    import heapq
    col_of = np.empty(N_NODES, dtype=np.int64)
    pos_of = np.empty(N_NODES, dtype=np.int64)
    for c in range(N_CORES):
        nodes = np.arange(c * NPC, (c + 1) * NPC)
        sorted_nodes = nodes[np.argsort(-deg[nodes], kind="stable")]
        heap = [(0, j) for j in range(NCOL)]  # (load, col); cols start empty
        counts = np.zeros(NCOL, dtype=np.int64)
        loads = np.zeros(NCOL, dtype=np.int64)
        spill = []
        degs = deg[sorted_nodes]
        for g, d in zip(sorted_nodes.tolist(), degs.tolist()):
            while True:
                load, j = heapq.heappop(heap)
                if counts[j] < NPCOL:
                    break
            col_of[g] = j
            pos_of[g] = counts[j]
            counts[j] += 1
            loads[j] = load + d
            if counts[j] < NPCOL:
                heapq.heappush(heap, (load + d, j))
        # repair pass: swap nodes out of any column above the slot cap
        it = 0
        while loads.max() > SLOTS_E:
            it += 1
            assert it < 5000, f"col overload {loads.max()}"
            hi = int(loads.argmax())
            lo = int(loads.argmin())
            hi_nodes = nodes[col_of[nodes] == hi]
            lo_nodes = nodes[col_of[nodes] == lo]
            a = hi_nodes[np.argmax(deg[hi_nodes])]
            b = lo_nodes[np.argmin(deg[lo_nodes])]
            if deg[a] <= deg[b]:
                raise RuntimeError("rebalance stuck")
            pa, pb = pos_of[a], pos_of[b]
            col_of[a], col_of[b] = lo, hi
            pos_of[a], pos_of[b] = pb, pa
            loads[hi] += deg[b] - deg[a]
            loads[lo] += deg[a] - deg[b]
        assert loads.max() <= SLOTS_E, f"col overload {loads.max()}"

    core_of = np.arange(N_NODES) // NPC
    perm_row = core_of * NLOC + col_of * NPCOL + pos_of  # global node -> table row

    # --- packed slot table: row | pos<<17 | qu<<20 ---
    EMPTY = np.uint32(ZROW | (7 << 17))
    idxp = np.full((N_CORES, 128, NCOL), EMPTY, dtype=np.uint32)

    # edge slots: per (core,col), nodes at pos 0..6 occupy consecutive slots
    deg_cp = np.zeros((N_CORES, NCOL, NPCOL), dtype=np.int64)
    deg_cp[core_of, col_of, pos_of] = deg
    start_cp = np.cumsum(deg_cp, axis=2) - deg_cp  # exclusive cumsum over pos
    slot_start = start_cp[core_of, col_of, pos_of]  # per node

    e_idx = np.arange(E, dtype=np.int64)
    within = e_idx - rowptr[s_dst]
    e_slot = slot_start[s_dst] + within
    e_core = core_of[s_dst]
    e_col = col_of[s_dst]
    qu = np.minimum(np.rint(s_u * UQ), UQ - 1).astype(np.uint32)
    packed = perm_row[s_src].astype(np.uint32) \
        | (pos_of[s_dst].astype(np.uint32) << 17) | (qu << 20)
    idxp[e_core, e_slot, e_col] = packed

    # self slots: slot 121+p gathers node's own row (pos=7, u=0 -> only the
    # constant self pattern column reads it)
    idxp[core_of, SLOTS_E + pos_of, col_of] = \
        perm_row.astype(np.uint32) | np.uint32(7 << 17)

    # --- per-node metadata in node-major chunk layout [128, n_chunks] ---
    nch = len(CH)
    deg_nm = np.zeros((N_CORES, 128 * nch), dtype=np.float32)
    batch_nm = np.full((N_CORES, 128 * nch), float(N_GRAPHS), dtype=np.float32)
    loc_row = col_of * NPCOL + pos_of
    deg_nm[core_of, loc_row] = 1.0 / deg_clip
    batch_nm[core_of, loc_row] = batch.astype(np.float32)
    deg_nm = deg_nm.reshape(N_CORES, nch, 128).transpose(0, 2, 1)
    batch_nm = batch_nm.reshape(N_CORES, nch, 128).transpose(0, 2, 1)

    # --- x table rows in local order, 4th channel = 1 (bias carrier) ---
    cnt = np.bincount(batch, minlength=N_GRAPHS).astype(np.float32)
    cnt_inv = (1.0 / np.maximum(cnt, 1.0)).astype(np.float32)

    return dict(idxp=idxp.view(np.int32), perm_row=perm_row,
                deg_nm=np.ascontiguousarray(deg_nm),
                batch_nm=np.ascontiguousarray(batch_nm),
                cnt_inv=cnt_inv, loc_row=loc_row, core_of=core_of)


def build_nc():
    nc = bacc.Bacc("TRN2", target_bir_lowering=False)

    idxp_in = nc.dram_tensor("idxp", [128, NCOL], i32, kind="ExternalInput")
    xloc_in = nc.dram_tensor("xloc", [NLOC, 4], bf16, kind="ExternalInput")
    deg_in = nc.dram_tensor("degnm", [128, len(CH)], f32, kind="ExternalInput")
    batchv_in = nc.dram_tensor("batchv", [128, len(CH)], f32, kind="ExternalInput")
    cntinv_in = nc.dram_tensor("cntinv", [128, 4], f32, kind="ExternalInput")
    fcw_in = nc.dram_tensor("fcw", [F, 8], f32, kind="ExternalInput")
    fcb_in = nc.dram_tensor("fcb", [128, 8], f32, kind="ExternalInput")
    wts_in = []
    for l in range(4):
        rows = 4 if l == 0 else F + 1
        wts_in.append(nc.dram_tensor(f"wpack_{l}", [rows, 3 * F], f32,
                                     kind="ExternalInput"))

    out_logits = nc.dram_tensor("out_logits", [N_GRAPHS, 8], f32,
                                kind="ExternalOutput")

    with tile.TileContext(nc) as tc:
        with (
            tc.tile_pool(name="res", bufs=1) as res,
            tc.tile_pool(name="gbuf", bufs=1) as gbuf,
            tc.tile_pool(name="pbuf", bufs=1) as pbuf,
            tc.tile_pool(name="ibuf", bufs=1) as ibuf,
            tc.tile_pool(name="sbuf_st", bufs=1) as stg,
            tc.tile_pool(name="work", bufs=2) as work,
            tc.tile_pool(name="psum_s", bufs=1, space="PSUM") as ps_s,
            tc.tile_pool(name="psum_d", bufs=1, space="PSUM") as ps_d,
            tc.tile_pool(name="psum_p", bufs=1, space="PSUM") as ps_p,
            tc.tile_pool(name="dram", bufs=1, space="DRAM") as dr,
        ):
            # ---------------- unpack slot table ----------------
            idxp_sb = res.tile([128, NCOL], i32)
            nc.sync.dma_start(idxp_sb[:], idxp_in[:])
            idx_sb = res.tile([128, NCOL], i32)
            nc.vector.tensor_scalar(out=idx_sb[:], in0=idxp_sb[:],
                                    scalar1=0x1FFFF, scalar2=None,
                                    op0=AL.bitwise_and)
            tmp_i = work.tile([128, NCOL], i32, tag="unp")
            nc.vector.tensor_scalar(out=tmp_i[:], in0=idxp_sb[:],
                                    scalar1=17, scalar2=7,
                                    op0=AL.logical_shift_right,
                                    op1=AL.bitwise_and)
            pos_f = res.tile([128, NCOL], f32)
            nc.vector.tensor_copy(pos_f[:], tmp_i[:])
            tmp_i2 = work.tile([128, NCOL], i32, tag="unp")
            nc.vector.tensor_scalar(out=tmp_i2[:], in0=idxp_sb[:],
                                    scalar1=20, scalar2=None,
                                    op0=AL.logical_shift_right)
            u_f = res.tile([128, NCOL], f32)
            nc.vector.tensor_copy(u_f[:], tmp_i2[:])
            u_bf = res.tile([128, NCOL], bf16)
            nc.vector.tensor_scalar(out=u_bf[:], in0=u_f[:], scalar1=1.0 / UQ,
                                    scalar2=None, op0=AL.mult)

            # ---------------- constants built on device ----------------
            iota7_i = res.tile([128, NPCOL], i32)
            nc.gpsimd.iota(iota7_i[:], pattern=[[1, NPCOL]], base=0,
                           channel_multiplier=0)
            iota7 = res.tile([128, NPCOL], f32)
            nc.vector.tensor_copy(iota7[:], iota7_i[:])
            selfp_i = res.tile([128, NPCOL], i32)
            nc.gpsimd.iota(selfp_i[:], pattern=[[-1, NPCOL]], base=-SLOTS_E,
                           channel_multiplier=1)
            selfpat = res.tile([128, NPCOL], bf16)
            nc.vector.tensor_scalar(out=selfpat[:], in0=selfp_i[:],
                                    scalar1=0, scalar2=None, op0=AL.is_equal)
            gids_i = res.tile([128, N_GRAPHS], i32)
            nc.gpsimd.iota(gids_i[:], pattern=[[1, N_GRAPHS]], base=0,
                           channel_multiplier=0)
            gids_f = res.tile([128, N_GRAPHS], f32)
            nc.vector.tensor_copy(gids_f[:], gids_i[:])

            # ---------------- small inputs ----------------
            deg_sb = res.tile([128, len(CH)], f32)
            nc.sync.dma_start(deg_sb[:], deg_in[:])
            batchv_sb = res.tile([128, len(CH)], f32)
            nc.sync.dma_start(batchv_sb[:], batchv_in[:])
            cntinv_sb = res.tile([128, 4], f32)
            nc.sync.dma_start(cntinv_sb[:], cntinv_in[:])
            fcw_sb = res.tile([F, 8], f32)
            nc.sync.dma_start(fcw_sb[:], fcw_in[:])
            fcb_sb = res.tile([128, 8], f32)
            nc.sync.dma_start(fcb_sb[:], fcb_in[:])
            w_sb = []
            for l in range(4):
                rows = 4 if l == 0 else F + 1
                t = res.tile([rows, 3 * F], f32, tag=f"w{l}")
                nc.sync.dma_start(t[:], wts_in[l][:])
                w_sb.append(t)

            # ---------------- aggregate buffers ----------------
            S_pl = res.tile([F, NLOC], f32)       # sum_j h_j        (transposed)
            S_u = res.tile([F, NLOC], f32)        # sum_j h_j * u    (transposed)
            S_rt = res.tile([F + 1, NLOC], f32)   # h_i (self); row F = ones
            nc.vector.memset(S_rt[F:F + 1, :], 1.0)

            x_tab = nc.dram_tensor("x_tab", [ZROW + 1, 4], bf16,
                                   kind="Internal", addr_space="Shared")
            h_tabs = [
                nc.dram_tensor(f"h_tab{i}", [ZROW + 1, F], bf16,
                               kind="Internal", addr_space="Shared")
                for i in range(2)
            ]
            ag_in = dr.tile([NLOC, F], bf16)
            pool_in = dr.tile([F, N_GRAPHS], f32)
            pool_out = dr.tile([F, N_GRAPHS], f32, addr_space="Shared")

            zrow = res.tile([1, F], bf16)
            nc.vector.memset(zrow[:], 0.0)
            nc.sync.dma_start(x_tab[ZROW:ZROW + 1, :], zrow[:, 0:4])
            for t in h_tabs:
                nc.sync.dma_start(t[ZROW:ZROW + 1, :], zrow[:])

            # gather x across cores (collectives can't read IO tensors
            # directly -> stage through an Internal DRAM buffer)
            xstage = dr.tile([NLOC, 4], bf16)
            nc.sync.dma_start(xstage[:], xloc_in[:])
            nc.gpsimd.collective_compute(
                "AllGather", AL.bypass,
                replica_groups=[list(range(N_CORES))],
                ins=[xstage.opt()],
                outs=[x_tab[0:ZROW, :].opt()],
            )

            pool_ps = ps_p.tile([F, N_GRAPHS], f32, space="PSUM")

            def scat_body(l, fin, table, c0, uu):
                """One bank group of the scatter stage; c0 may be symbolic."""
                idx_st = ibuf.tile([128, BG], i32, tag=f"ist{uu}")
                nc.vector.tensor_copy(idx_st[:], idx_sb[:, ds(c0, BG)])
                g_t = gbuf.tile([128, BG, fin], bf16, tag=f"g{min(l, 1)}_{uu}")
                for c in range(BG):
                    nc.gpsimd.indirect_dma_start(
                        out=g_t[:, c, :], out_offset=None, in_=table,
                        in_offset=bass.IndirectOffsetOnAxis(
                            ap=idx_st[:, c:c + 1], axis=0),
                    )
                p_t = pbuf.tile([128, BG, NPCOL, 3], bf16, tag=f"pat{uu}")
                nc.vector.tensor_tensor(
                    out=p_t[:, :, :, 0],
                    in0=pos_f[:, ds(c0, BG)].unsqueeze(2)
                        .to_broadcast([128, BG, NPCOL]),
                    in1=iota7[:].unsqueeze(1).to_broadcast([128, BG, NPCOL]),
                    op=AL.is_equal)
                nc.vector.tensor_tensor(
                    out=p_t[:, :, :, 1],
                    in0=p_t[:, :, :, 0],
                    in1=u_bf[:, ds(c0, BG)].unsqueeze(2)
                        .to_broadcast([128, BG, NPCOL]),
                    op=AL.mult)
                nc.vector.tensor_copy(
                    p_t[:, :, :, 2],
                    selfpat[:].unsqueeze(1).to_broadcast([128, BG, NPCOL]))
                bank = ps_s.tile([F, BG * PWC], f32, tag=f"scat{uu}",
                                 space="PSUM")
                for c in range(BG):
                    nc.tensor.matmul(
                        bank[0:fin, c * PWC:(c + 1) * PWC],
                        lhsT=g_t[:, c, :],
                        rhs=p_t[:, c].rearrange("p k t -> p (k t)"),
                        start=True, stop=True,
                    )
                bview = bank[0:fin].rearrange("f (c k t) -> f t (c k)",
                                              k=NPCOL, t=3)
                dst = ds(c0 * NPCOL, BG * NPCOL)
                nc.vector.tensor_copy(S_pl[0:fin, dst], bview[:, 0, :])
                nc.vector.tensor_copy(S_u[0:fin, dst], bview[:, 1, :])
                nc.vector.tensor_copy(S_rt[0:fin, dst], bview[:, 2, :])

            def dense_chunk(l, fin, rr, w_t, t0, n, k_idx, uu,
                            symbolic):
                """Dense + deg scale + ELU for nodes [t0, t0+n)."""
                if symbolic:
                    spl = stg.tile([F, 128], f32, tag=f"spl{uu}")
                    nc.vector.tensor_copy(spl[0:fin, 0:n],
                                          S_pl[0:fin, ds(t0, n)])
                    su = stg.tile([F, 128], f32, tag=f"su{uu}")
                    nc.vector.tensor_copy(su[0:fin, 0:n],
                                          S_u[0:fin, ds(t0, n)])
                    srt = stg.tile([F + 1, 128], f32, tag=f"srt{uu}")
                    nc.vector.tensor_copy(srt[0:rr, 0:n],
                                          S_rt[0:rr, ds(t0, n)])
                    spl_ap, su_ap, srt_ap = (spl[0:fin, 0:n], su[0:fin, 0:n],
                                             srt[0:rr, 0:n])
                    degc = deg_sb[0:n, ds(k_idx, 1)]
                else:
                    spl_ap = S_pl[0:fin, t0:t0 + n]
                    su_ap = S_u[0:fin, t0:t0 + n]
                    srt_ap = S_rt[0:rr, t0:t0 + n]
                    degc = deg_sb[0:n, k_idx:k_idx + 1]
                d_ps = ps_d.tile([128, 128], f32, tag=f"dense{uu}",
                                 space="PSUM")
                nc.tensor.matmul(d_ps[0:n, 0:F], lhsT=spl_ap,
                                 rhs=w_t[0:fin, 0:F], start=True, stop=False)
                nc.tensor.matmul(d_ps[0:n, 0:F], lhsT=su_ap,
                                 rhs=w_t[0:fin, F:2 * F], start=False,
                                 stop=True)
                nc.tensor.matmul(d_ps[0:n, F:2 * F], lhsT=srt_ap,
                                 rhs=w_t[0:rr, 2 * F:3 * F], start=True,
                                 stop=True)
                z_t = work.tile([128, F], f32, tag=f"z{uu}")
                nc.vector.tensor_scalar(
                    out=z_t[0:n, :], in0=d_ps[0:n, 0:F],
                    scalar1=degc, scalar2=None, op0=AL.mult)
                nc.vector.tensor_tensor(
                    out=z_t[0:n, :], in0=z_t[0:n, :],
                    in1=d_ps[0:n, F:2 * F], op=AL.add)
                # ELU(z) = max(z, min(exp(z),1) - 1)
                ex_t = work.tile([128, F], f32, tag=f"ex{uu}")
                nc.scalar.activation(ex_t[0:n, :], z_t[0:n, :], ACTF.Exp)
                nc.vector.tensor_scalar(
                    out=ex_t[0:n, :], in0=ex_t[0:n, :],
                    scalar1=1.0, scalar2=-1.0, op0=AL.min, op1=AL.add)
                h_t = work.tile([128, F], bf16 if l < 3 else f32,
                                tag=f"h{uu}_{l < 3}")
                nc.vector.tensor_tensor(
                    out=h_t[0:n, :], in0=z_t[0:n, :], in1=ex_t[0:n, :],
                    op=AL.max)
                return h_t

            NFULL = (NLOC // 128) * 128  # 9984

            for l in range(4):
                fin = 4 if l == 0 else F
                rr = 4 if l == 0 else F + 1  # root matmul contraction rows
                w_t = w_sb[l]
                if l == 0:
                    table = x_tab[:]
                else:
                    table = h_tabs[(l - 1) % 2][:]

                # --- scatter: gather + on-device pattern + matmuls ---
                with tc.For_i(0, NCOL, BG * 3, staggered_reset=True) as i0:
                    for uu in range(3):
                        scat_body(l, fin, table, i0 + uu * BG, uu)

                # --- dense (node-major out) + deg scale + ELU ---
                if l < 3:
                    with tc.For_i(0, NFULL, 256, staggered_reset=True) as i0:
                        for uu in range(2):
                            t0 = i0 + uu * 128
                            h_t = dense_chunk(l, fin, rr, w_t, t0, 128,
                                              t0 // 128, uu, True)
                            nc.sync.dma_start(ag_in[ds(t0, 128), :], h_t[:])
                    # tail chunk
                    n = NLOC - NFULL
                    h_t = dense_chunk(l, fin, rr, w_t, NFULL, n,
                                      NFULL // 128, 0, False)
                    nc.sync.dma_start(ag_in[NFULL:NLOC, :], h_t[0:n, :])
                    nc.gpsimd.collective_compute(
                        "AllGather", AL.bypass,
                        replica_groups=[list(range(N_CORES))],
                        ins=[ag_in.opt()],
                        outs=[h_tabs[l % 2][0:ZROW, :].opt()],
                    )
                else:
                    for k, (t0, t1) in enumerate(CH):
                        n = t1 - t0
                        h_t = dense_chunk(l, fin, rr, w_t, t0, n, k,
                                          k % 2, False)
                        if n < 128:
                            nc.vector.memset(h_t[n:128, :], 0.0)
                        oh_t = work.tile([128, N_GRAPHS], f32, tag="oh")
                        nc.vector.tensor_scalar(
                            out=oh_t[:], in0=gids_f[:],
                            scalar1=batchv_sb[:, k:k + 1], scalar2=None,
                            op0=AL.is_equal)
                        nc.tensor.matmul(
                            pool_ps[:], lhsT=h_t[:], rhs=oh_t[:],
                            start=(k == 0), stop=(k == len(CH) - 1))

            # ---------------- pooling all-reduce + head ----------------
            pool_sb = res.tile([F, N_GRAPHS], f32)
            nc.vector.tensor_copy(pool_sb[:], pool_ps[:])
            nc.sync.dma_start(pool_in[:], pool_sb[:])
            nc.gpsimd.collective_compute(
                "AllReduce", AL.add,
                replica_groups=[list(range(N_CORES))],
                ins=[pool_in.opt()], outs=[pool_out.opt()],
            )
            pooled = res.tile([F, N_GRAPHS], f32)
            nc.sync.dma_start(pooled[:], pool_out[:])

            for gch in range(N_GRAPHS // 128):
                g0 = gch * 128
                l_ps = ps_p.tile([128, 8], f32, tag="head", space="PSUM")
                nc.tensor.matmul(
                    l_ps[:, 0:8], lhsT=pooled[:, g0:g0 + 128], rhs=fcw_sb[:],
                    start=True, stop=True)
                z_t = work.tile([128, 8], f32, tag="hz")
                nc.vector.tensor_scalar(
                    out=z_t[:], in0=l_ps[:],
                    scalar1=cntinv_sb[:, gch:gch + 1], scalar2=None,
                    op0=AL.mult)
                nc.vector.tensor_tensor(out=z_t[:], in0=z_t[:], in1=fcb_sb[:],
                                        op=AL.add)
                rm = work.tile([128, 1], f32, tag="rm")
                nc.vector.tensor_reduce(rm[:], z_t[:, 0:6], axis=AX.X, op=AL.max)
                zs = work.tile([128, 8], f32, tag="zs")
                nc.vector.tensor_scalar(
                    out=zs[:], in0=z_t[:], scalar1=rm[:], scalar2=None,
                    op0=AL.subtract)
                e_t = work.tile([128, 8], f32, tag="et")
                nc.scalar.activation(e_t[:, 0:6], zs[:, 0:6], ACTF.Exp)
                sm = work.tile([128, 1], f32, tag="sm")
                nc.vector.tensor_reduce(sm[:], e_t[:, 0:6], axis=AX.X, op=AL.add)
                ln = work.tile([128, 1], f32, tag="ln")
                nc.scalar.activation(ln[:], sm[:], ACTF.Ln)
                oT = work.tile([128, 8], f32, tag="oT")
                nc.vector.tensor_scalar(
                    out=oT[:], in0=zs[:], scalar1=ln[:], scalar2=None,
                    op0=AL.subtract)
                nc.sync.dma_start(out_logits[g0:g0 + 128, :], oT[:])

    nc.compile()
    return nc


def make_in_maps(plan, x, weights):
    import ml_dtypes
    x = np.asarray(x, dtype=np.float32)
    perm_row, loc_row, core_of = plan["perm_row"], plan["loc_row"], plan["core_of"]

    xloc = np.zeros((N_CORES, NLOC, 4), dtype=np.float32)
    xloc[core_of, loc_row, 0:3] = x
    xloc[core_of, loc_row, 3] = 1.0
    xloc = xloc.astype(ml_dtypes.bfloat16)

    fcb = np.zeros((128, 8), dtype=np.float32)
    fcb[:, :6] = np.asarray(weights["fc_b"], dtype=np.float32)
    fcw = np.zeros((F, 8), dtype=np.float32)
    fcw[:, :6] = np.asarray(weights["fc_w"], dtype=np.float32)
    cntinv = np.ascontiguousarray(
        plan["cnt_inv"].reshape(4, 128).T).astype(np.float32)

    wps = []
    for l in range(4):
        W = np.asarray(weights[f"W{l+1}"], dtype=np.float32)
        root = np.asarray(weights[f"root{l+1}"], dtype=np.float32)
        b = np.asarray(weights[f"b{l+1}"], dtype=np.float32)
        rows = 4 if l == 0 else F + 1
        wp = np.zeros((rows, 3 * F), np.float32)
        fin_d = W.shape[1]  # 3 or 64
        wp[:fin_d, 0:F] = W[0]
        wp[:fin_d, F:2 * F] = W[1] - W[0]
        wp[:fin_d, 2 * F:3 * F] = root
        wp[rows - 1, 2 * F:3 * F] = b  # bias rides the ones row
        wps.append(wp)

    in_maps = []
    for c in range(N_CORES):
        im = {
            "idxp": plan["idxp"][c],
            "xloc": xloc[c],
            "degnm": plan["deg_nm"][c],
            "batchv": plan["batch_nm"][c],
            "cntinv": cntinv,
            "fcw": fcw, "fcb": fcb,
        }
        for l in range(4):
            im[f"wpack_{l}"] = wps[l]
        in_maps.append(im)
    return in_maps


_NC_CACHE = {}


def kernel(**inputs):
    x = np.asarray(inputs["x"], dtype=np.float32)
    pseudo = np.asarray(inputs["pseudo"], dtype=np.float32)
    edge_index = np.asarray(inputs["edge_index"]).astype(np.int64)
    batch = np.asarray(inputs["batch"]).astype(np.int64)
    weights = {k: np.asarray(inputs[k], dtype=np.float32) for k in
               ["W1", "root1", "b1", "W2", "root2", "b2", "W3", "root3",
                "b3", "W4", "root4", "b4", "fc_w", "fc_b"]}

    plan = build_plan(edge_index, pseudo, batch)
    in_maps = make_in_maps(plan, x, weights)

    if "nc" not in _NC_CACHE:
        _NC_CACHE["nc"] = build_nc()
    nc = _NC_CACHE["nc"]

    res = run_bass_kernel_spmd(nc, in_maps, core_ids=list(range(N_CORES)))
    return np.ascontiguousarray(res.results[0]["out_logits"][:, :6]).astype(np.float32)
